# revision 68
# baseline (speedup 1.0000x reference)
"""Differential attention (dense_transformer) Trainium2 kernel.

Full-input contract: kernel(**inputs) takes the unsharded inputs of
reference.setup_inputs() and returns the full (1, S, D) float32 output.

Sharding: 16 heads across 8 cores (2 heads/core, tensor-parallel on the
q/k/v projection rows and wo columns). Each core computes a full (S, D)
partial of the output projection; the host sums partials and adds the
residual.
"""

import sys

for _p in ("/opt/trn_rl_repo", "/root/.axon_site/_ro/trn_rl_repo"):
    if _p not in sys.path:
        sys.path.insert(0, _p)

import math

import numpy as np

import concourse.bass as bass
import concourse.mybir as mybir
import concourse.tile as tile
from concourse import bacc
from concourse.bass import ts
from concourse.bass_utils import run_bass_kernel_spmd
from concourse.masks import make_identity, make_upper_triangular

F32 = mybir.dt.float32
F16 = mybir.dt.float16

# Problem constants
B, S, D = 1, 2048, 2048
H, C, HD = 16, 2, 128
DM = HD * C  # 256 per-head q/k dim
N_CORES = 8
HPC = H // N_CORES  # heads per core = 2
NHC = HPC * C  # head-comp blocks per core = 4
EPS = 1e-9
CONST = 10000.0
SQ = 512  # S_q super-tile width

# pool-size knobs (model-tuned)
CFG = {"ph1": 3, "pps": 5, "vpps": 3, "spp": 2, "ep": 10, "qkps": 2,
       "avps": 2, "ost": 3, "comb": 2, "attc": 2}


def build_kernel(s=S, loop_n=1):
    """Build the per-core Bass kernel (SPMD; per-core data differs).

    loop_n > 1 wraps the whole body in a hardware loop (timing only)."""
    import contextlib

    ns = s // 128  # S chunks of 128
    nj = s // SQ  # S_q super tiles
    kd = D // 128  # contraction chunks over D

    nc = bacc.Bacc("TRN2", target_bir_lowering=False, debug=False,
                   num_devices=N_CORES)

    x_d = nc.dram_tensor("x", [s, D], F16, kind="ExternalInput")
    xtr_d = nc.dram_tensor("xtr", [D, s], F16, kind="ExternalInput")
    wqt_d = nc.dram_tensor("wqt", [D, NHC * 128], F16, kind="ExternalInput")
    wkt_d = nc.dram_tensor("wkt", [D, NHC * 128], F16, kind="ExternalInput")
    wvt_d = nc.dram_tensor("wvt", [D, HPC * HD], F16, kind="ExternalInput")
    wot_d = nc.dram_tensor("wot", [HPC * HD, D], F16, kind="ExternalInput")
    cost_d = nc.dram_tensor("cost", [128, s], F16, kind="ExternalInput")
    sint_d = nc.dram_tensor("sint", [128, s], F16, kind="ExternalInput")
    lam_d = nc.dram_tensor("lam", [1, HPC], F32, kind="ExternalInput")
    out_d = nc.dram_tensor("out", [s, D], F16, kind="ExternalOutput")

    inv_sqrt_hd = 1.0 / math.sqrt(HD)
    I32 = mybir.dt.int32
    # float32 whose bit pattern is the rsqrt magic constant 0x5f3759df
    RSQRT_MAGIC = float(np.frombuffer(np.uint32(0x5F3759DF).tobytes(),
                                      np.float32)[0])

    def emit_rsqrt(out_f32, m_f32, ytile, ttile, ktile, shape, eng=None):
        """out = m^-0.5 via bit-trick seed + 2 Newton steps (no tables).
        ytile/ttile are f32 scratch APs of `shape`; ktile holds the magic."""
        eng = eng or nc.vector
        mul = mybir.AluOpType.mult
        eng.tensor_scalar(
            out=ytile.bitcast(I32), in0=m_f32.bitcast(I32), scalar1=1,
            scalar2=None, op0=mybir.AluOpType.logical_shift_right)
        eng.tensor_tensor(
            out=ytile.bitcast(I32), in0=ktile.bitcast(I32).to_broadcast(shape),
            in1=ytile.bitcast(I32), op=mybir.AluOpType.subtract)
        for it in range(2):
            tgt = out_f32 if it == 1 else ytile
            eng.tensor_tensor(out=ttile, in0=ytile, in1=ytile, op=mul)
            eng.tensor_tensor(out=ttile, in0=ttile, in1=m_f32, op=mul)
            eng.tensor_scalar(out=ttile, in0=ttile, scalar1=-0.5,
                              scalar2=1.5, op0=mul,
                              op1=mybir.AluOpType.add)
            eng.tensor_tensor(out=tgt, in0=ytile, in1=ttile, op=mul)

    with tile.TileContext(nc) as tc:
        with (
            (tc.For_i(0, loop_n, 1) if loop_n > 1
             else contextlib.nullcontext()),
            tc.tile_pool(name="const", bufs=1) as cp,
            tc.tile_pool(name="qk", bufs=1) as qkp,
            tc.tile_pool(name="vat", bufs=1) as vap,
        ):
            # ---- small persistent constants ----
            lam = cp.tile([128, HPC], F32, tag="lam")
            m0 = cp.tile([128, 128], F16, tag="m0")
            make_upper_triangular(nc, m0, val=1.0, diag=True)
            ident = cp.tile([128, 128], F16, tag="ident")
            make_identity(nc, ident)
            ktile = cp.tile([128, 1], F32, tag="ktile")
            nc.vector.memset(ktile, RSQRT_MAGIC)

            # persistent activations (split into dependency-granular tiles
            # so consumers start as soon as their slice is ready)
            qTs = [qkp.tile([128, s], F16, tag=f"qT{hc}", name=f"qT{hc}")
                   for hc in range(NHC)]
            kTs = [qkp.tile([128, s], F16, tag=f"kT{hc}", name=f"kT{hc}")
                   for hc in range(NHC)]
            # vaug per (head, S-quarter): [128, 4, 132]
            vaug = [[vap.tile([128, 4, 132], F16, tag=f"va{h}_{q}", name=f"va{h}_{q}")
                     for q in range(nj)] for h in range(HPC)]

            with (
                tc.tile_pool(name="wqkv", bufs=1) as wp,
                tc.tile_pool(name="ht", bufs=1) as htp,
            ):
                wqt = wp.tile([128, kd, NHC * 128], F16, tag="wqt")
                wkt = wp.tile([128, kd, NHC * 128], F16, tag="wkt")
                wvt = wp.tile([128, kd, HPC * HD], F16, tag="wvt")
                # hT split into S-quarters so projections of quarter j only
                # wait on that quarter's 4 transposes
                hts = [htp.tile([128, kd, SQ], F16, tag=f"ht{q}", name=f"ht{q}")
                       for q in range(nj)]

                # ---- phase 1: hT loads (host-pre-transposed x) + stats ----
                # The rmsnorm scale r is deferred off the critical path:
                # r multiplies q/k exactly via the (linear) RoPE tables and
                # v via its per-partition psum eviction. hT arrives as four
                # plain quarter loads of the host-transposed x; the natural
                # x tiles feed only the stats side-path.
                nc.sync.dma_start(out=wvt,
                                  in_=wvt_d.rearrange("(k p) m -> p k m", p=128))
                stats = wp.tile([128, ns], F32, tag="stats")
                rcol = wp.tile([128, ns], F32, tag="rcol")
                xtr_ap = xtr_d.rearrange("(k p) m -> p k m", p=128)
                with (
                    tc.tile_pool(name="ph1", bufs=4) as p1,
                    tc.tile_pool(name="ph1b", bufs=2) as p1b,
                    tc.tile_pool(name="rsc", bufs=2) as rscp,
                ):
                    def stat_group(g):
                        xts = []
                        for i in range(4 * g, 4 * g + 4):
                            xt = p1.tile([128, D], F16, tag="xt")
                            xts.append(xt)
                            nc.sync.dma_start(out=xt, in_=x_d[ts(i, 128), :])
                        for di, i in enumerate(range(4 * g, 4 * g + 4)):
                            sq = p1b.tile([128, D], F16, tag="sq")
                            nc.vector.scalar_tensor_tensor(
                                out=sq, in0=xts[di], scalar=1.0, in1=xts[di],
                                op0=mybir.AluOpType.mult,
                                op1=mybir.AluOpType.mult,
                                accum_out=stats[:, i:i + 1])
                        msl = stats[:, 4 * g:4 * g + 4]
                        nc.vector.tensor_scalar(
                            out=msl, in0=msl, scalar1=1.0 / D, scalar2=EPS,
                            op0=mybir.AluOpType.mult, op1=mybir.AluOpType.add)
                        ysc = rscp.tile([128, 4], F32, tag="ysc")
                        tsc = rscp.tile([128, 4], F32, tag="tsc")
                        emit_rsqrt(rcol[:, 4 * g:4 * g + 4], msl, ysc, tsc,
                                   ktile, (128, 4))

                    # interleave: each hT quarter unlocks projection work,
                    # its stats group unlocks the v evictions; q/k weight
                    # chunks slot in after the first/second quarter
                    for g in range(nj):
                        nc.sync.dma_start(out=hts[g],
                                          in_=xtr_ap[:, :, ts(g, SQ)])
                        stat_group(g)
                        if g == 0 or nj == 1:
                            for k in range(kd):
                                nc.sync.dma_start(out=wkt[:, k, :],
                                                  in_=wkt_d[ts(k, 128), :])
                        if g == 1 or nj == 1:
                            for k in range(kd):
                                nc.sync.dma_start(out=wqt[:, k, :],
                                                  in_=wqt_d[ts(k, 128), :])

                # ---- phase 2: projections + RoPE + repack ----
                # q/k are projected into a "split" row layout
                # [R0, R1, I0, I1] (R = rope-real rows, I = rope-imag rows;
                # j2 in {0,1} indexes the two 128-row groups of real parts).
                # RoPE then runs full-lane with partition-aligned operands,
                # and SBUF->SBUF DMAs repack into per-head-comp [xr;xi]
                # tiles (qT/kT) for K=128 attention matmuls.
                mul = mybir.AluOpType.mult
                with tc.tile_pool(name="pps", bufs=CFG["pps"], space="PSUM") as pps, \
                     tc.tile_pool(name="vpps", bufs=CFG["vpps"], space="PSUM") as vpps, \
                     tc.tile_pool(name="split", bufs=CFG["spp"]) as spp, \
                     tc.tile_pool(name="rope", bufs=1) as rp, \
                     tc.tile_pool(name="rdp", bufs=1, space="DRAM") as rdp, \
                     tc.tile_pool(name="ropec", bufs=1) as rcp:
                    cost = rcp.tile([128, s], F16, tag="cost")
                    nc.sync.dma_start(out=cost, in_=cost_d[:, :])
                    sint = rcp.tile([128, s], F16, tag="sint")
                    nc.sync.dma_start(out=sint, in_=sint_d[:, :])
                    # v first so attention's AV operands are ready early;
                    # the deferred rmsnorm scale rides the psum eviction
                    for i in range(ns):
                        ps = vpps.tile([128, HPC * HD], F32, tag="vps")
                        for k in range(kd):
                            nc.tensor.matmul(ps, hts[i // 4][:, k, ts(i % 4, 128)],
                                             wvt[:, k, :],
                                             start=(k == 0), stop=(k == kd - 1))
                        for h in range(HPC):
                            nc.vector.tensor_scalar_mul(
                                out=vaug[h][i // 4][:, i % 4, 0:128],
                                in0=ps[:, ts(h, 128)],
                                scalar1=rcol[:, i:i + 1])
                    for h in range(HPC):
                        for q in range(nj):
                            nc.vector.memset(vaug[h][q][:, :, 128:129], 1.0)
                    # fold r into the rope tables: cos/sin *= r[s] along the
                    # free axis (broadcast r via a DRAM bounce)
                    rc16g = rcp.tile([128, ns], F16, tag="rc16g")
                    nc.vector.tensor_copy(out=rc16g, in_=rcol)
                    rd = rdp.tile([1, s], F16, tag="rd")
                    nc.sync.dma_start(
                        out=rd[0:1, :].rearrange("o (i p) -> o p i", p=128),
                        in_=rc16g)
                    rbc = rcp.tile([128, s], F16, tag="rbc")
                    _rdap = rd[0:1, :]
                    nc.sync.dma_start(
                        out=rbc,
                        in_=bass.AP(tensor=_rdap.tensor, offset=_rdap.offset,
                                    ap=[[0, 128]] + list(_rdap.ap)[1:]))
                    nc.vector.tensor_tensor(out=cost, in0=cost, in1=rbc,
                                            op=mybir.AluOpType.mult)
                    nc.vector.tensor_tensor(out=sint, in0=sint, in1=rbc,
                                            op=mybir.AluOpType.mult)
                    # per (j2, tensor): project the (R_j2, I_j2) pair, RoPE,
                    # repack -- so head j2's attention can start while the
                    # other head is still projecting
                    for j2 in range(2):
                        for w_sb, t_sbs in ((wkt, kTs), (wqt, qTs)):
                            qs2 = spp.tile([128, 2, s], F16, tag="qs")
                            # quarter-major so each hT quarter unlocks both
                            # row-groups' matmuls as soon as it lands
                            for j in range(nj):
                                for mbi, mb in enumerate((j2, j2 + 2)):
                                    ps = pps.tile([128, SQ], F32, tag="ps")
                                    for k in range(kd):
                                        nc.tensor.matmul(
                                            ps, w_sb[:, k, ts(mb, 128)],
                                            hts[j][:, k, :],
                                            start=(k == 0), stop=(k == kd - 1))
                                    nc.scalar.activation(
                                        out=qs2[:, mbi, ts(j, SQ)], in_=ps,
                                        func=mybir.ActivationFunctionType.Copy)
                            # RoPE in place, full 128 lanes
                            xr = qs2[:, 0, :]
                            xi = qs2[:, 1, :]
                            t2 = rp.tile([128, s], F16, tag="t2")
                            t3 = rp.tile([128, s], F16, tag="t3")
                            nc.vector.tensor_tensor(out=t2, in0=xi, in1=sint,
                                                    op=mul)
                            nc.vector.tensor_tensor(out=t3, in0=xr, in1=sint,
                                                    op=mul)
                            nc.vector.tensor_tensor(out=xr, in0=xr, in1=cost,
                                                    op=mul)
                            nc.vector.tensor_tensor(out=xr, in0=xr, in1=t2,
                                                    op=mybir.AluOpType.subtract)
                            nc.vector.tensor_tensor(out=xi, in0=xi, in1=cost,
                                                    op=mul)
                            nc.vector.tensor_tensor(out=xi, in0=xi, in1=t3,
                                                    op=mybir.AluOpType.add)
                            # repack: hc tile = [xr(64) ; xi(64)]
                            for half in range(2):
                                hc = 2 * j2 + half
                                nc.sync.dma_start(
                                    out=t_sbs[hc][0:64, :],
                                    in_=qs2[ts(half, 64), 0, :])
                                nc.sync.dma_start(
                                    out=t_sbs[hc][64:128, :],
                                    in_=qs2[ts(half, 64), 1, :])

            # ---- phase 3 + 4: attention then output projection ----
            # attT per (head, S_q super-tile) so the output projection can
            # start on a row range as soon as both heads' combines finish.
            attT = [[qkp.tile([128, SQ], F16, tag=f"attT{h}_{q}", name=f"attT{h}_{q}")
                     for q in range(nj)] for h in range(HPC)]
            with (
                tc.tile_pool(name="ep", bufs=CFG["ep"]) as ep,
                tc.tile_pool(name="qkps", bufs=CFG["qkps"], space="PSUM") as qkps,
                tc.tile_pool(name="avps", bufs=CFG["avps"], space="PSUM") as avps,
                tc.tile_pool(name="tpps", bufs=1, space="PSUM") as tpps,
                tc.tile_pool(name="comb", bufs=CFG["comb"]) as cbp,
                tc.tile_pool(name="attc", bufs=CFG["attc"]) as atcp,
                tc.tile_pool(name="small", bufs=8) as smp,
                tc.tile_pool(name="wo", bufs=1) as wop,
                tc.tile_pool(name="ops", bufs=1, space="PSUM") as opsp,
                tc.tile_pool(name="ost", bufs=CFG["ost"]) as ostp,
            ):
                wot = wop.tile([128, HPC, D], F16, tag="wot")
                nc.sync.dma_start(out=wot,
                                  in_=wot_d.rearrange("(h p) n -> p h n", p=128))
                _lap = lam_d[:, :]
                nc.sync.dma_start(
                    out=lam,
                    in_=bass.AP(tensor=_lap.tensor, offset=_lap.offset,
                                ap=[[0, 128]] + list(_lap.ap)[1:]),
                )
                mul = mybir.AluOpType.mult
                add = mybir.AluOpType.add
                for j in range(nj - 1, -1, -1):
                    for head in range(HPC):
                        avsb = []
                        for c2 in range(C):
                            hc = C * head + c2
                            nblk = 4 * j + 4
                            # fused softmax normalization targets
                            attn_c = atcp.tile([128, 4, 128], F16,
                                               tag=f"attn{c2}", name=f"attn{c2}")
                            drc = smp.tile([128, 4, 1], F32, tag=f"drc{c2}",
                                           name=f"drc{c2}")
                            avsb.append(attn_c)
                            es = []
                            # S_k blocks in pairs: one 2-bank PSUM tile and
                            # (off-diagonal) one wide Exp per pair
                            for i2 in range(0, nblk, 2):
                                eps2 = qkps.tile([128, 2, SQ], F32, tag="eps")
                                et2 = ep.tile([128, 2, SQ], F16, tag="et")
                                diag = i2 >= 4 * j
                                for di in range(2):
                                    i = i2 + di
                                    c0 = 128 * max(i - 4 * j, 0)
                                    nc.tensor.matmul(
                                        eps2[:, di, c0:SQ], kTs[hc][:, ts(i, 128)],
                                        qTs[hc][:, SQ * j + c0:SQ * j + SQ],
                                        start=True, stop=True)
                                    if diag:
                                        nc.scalar.activation(
                                            out=et2[:, di, c0:SQ],
                                            in_=eps2[:, di, c0:SQ],
                                            func=mybir.ActivationFunctionType.Exp,
                                            scale=inv_sqrt_hd)
                                        nc.vector.tensor_tensor(
                                            out=et2[:, di, c0:c0 + 128],
                                            in0=et2[:, di, c0:c0 + 128],
                                            in1=m0, op=mul)
                                if not diag:
                                    nc.scalar.activation(
                                        out=et2, in_=eps2,
                                        func=mybir.ActivationFunctionType.Exp,
                                        scale=inv_sqrt_hd)
                                es.append(et2)
                            for m in range(4):
                                avm = avps.tile([128, 129], F32, tag="avm")
                                for i in range(4 * j + m + 1):
                                    nc.tensor.matmul(
                                        avm, es[i // 2][:, i % 2, ts(m, 128)],
                                        vaug[head][i // 4][:, i % 4, 0:129],
                                        start=(i == 0), stop=(i == 4 * j + m))
                                nc.vector.reciprocal(out=drc[:, m, :],
                                                     in_=avm[:, 128:129])
                                nc.vector.tensor_scalar_mul(
                                    out=attn_c[:, m, :], in0=avm[:, 0:128],
                                    scalar1=drc[:, m, :])
                        # combine components + head RMSNorm (f16, 2x mode)
                        comb = cbp.tile([128, 4, 128], F16, tag="comb")
                        nc.vector.scalar_tensor_tensor(
                            out=comb, in0=avsb[1], scalar=lam[:, head:head + 1],
                            in1=avsb[0], op0=mul, op1=add)
                        tt = cbp.tile([128, 4, 128], F16, tag="tt")
                        nc.vector.tensor_tensor(out=tt, in0=comb, in1=comb, op=mul)
                        ssum = smp.tile([128, 4, 1], F32, tag="ssum")
                        nc.vector.reduce_sum(out=ssum, in_=tt,
                                             axis=mybir.AxisListType.X)
                        nc.vector.tensor_scalar(
                            out=ssum, in0=ssum, scalar1=1.0 / HD, scalar2=EPS,
                            op0=mul, op1=add)
                        rf = smp.tile([128, 4, 1], F32, tag="rf")
                        ycb = smp.tile([128, 4, 1], F32, tag="ycb")
                        tcb = smp.tile([128, 4, 1], F32, tag="tcb")
                        emit_rsqrt(rf, ssum, ycb, tcb,
                                   ktile[:, :, None], (128, 4, 1))
                        a16 = cbp.tile([128, 4, 128], F16, tag="a16")
                        nc.vector.tensor_tensor(
                            out=a16, in0=comb, in1=rf.to_broadcast((128, 4, 128)),
                            op=mul)
                        for mm in range(4):
                            tp = tpps.tile([128, 128], F16, tag="tp")
                            nc.tensor.transpose(tp, a16[:, mm, :], ident)
                            nc.vector.tensor_copy(
                                out=attT[head][j][:, ts(mm, 128)], in_=tp)
                        if head == HPC - 1:
                            # output projection for this super-tile's rows
                            for sm in range(4 * j, 4 * j + 4):
                                for dn in range(D // SQ):
                                    ps = opsp.tile([128, SQ], F32, tag="ops")
                                    for h in range(HPC):
                                        nc.tensor.matmul(
                                            ps,
                                            attT[h][sm // 4][:, ts(sm % 4, 128)],
                                            wot[:, h, ts(dn, SQ)],
                                            start=(h == 0), stop=(h == HPC - 1))
                                    ost = ostp.tile([128, SQ], F16, tag="ost")
                                    if j <= 1:
                                        nc.scalar.activation(
                                            out=ost, in_=ps,
                                            func=mybir.ActivationFunctionType.Copy)
                                    else:
                                        nc.vector.tensor_copy(out=ost, in_=ps)
                                    nc.sync.dma_start(
                                        out=out_d[ts(sm, 128), ts(dn, SQ)],
                                        in_=ost)

    nc.compile()
    return nc


def _perm_core():
    """Row permutation of one core's HPC*DM q/k rows into the split layout
    [R0..R_{HPC-1}, I0..I_{HPC-1}]: R_h = rope-real (even) rows of head h for
    both components, I_h = rope-imag (odd) rows. Within each 128-row block,
    rows follow theta-pair order 0..127."""
    evens = [h * DM + 128 * c + 2 * t
             for h in range(HPC) for c in range(C) for t in range(64)]
    odds = [h * DM + 128 * c + 2 * t + 1
            for h in range(HPC) for c in range(C) for t in range(64)]
    return np.array(evens + odds)


def prep_inputs(x, pre_norm_w, wq, wk, wv, wo, head_norm_w, q1, q2, k1, k2,
                lam_init, s=S):
    """Host-side prep: fold norms/lambdas into weights, permute q/k rows,
    transpose, cast fp16, build rope tables; returns per-core input maps."""
    x2 = np.asarray(x, np.float32).reshape(s, D)
    pw = np.asarray(pre_norm_w, np.float32)
    hw = np.asarray(head_norm_w, np.float32)
    li = np.asarray(lam_init, np.float64)

    wq_e = (np.asarray(wq, np.float64) * pw[None, :])
    wk_e = (np.asarray(wk, np.float64) * pw[None, :])
    wv_e = (np.asarray(wv, np.float64) * pw[None, :])
    # wo: out = att_normed * (1-lam) @ wo.T ; head_norm_w folds per att dim
    colscale = np.concatenate(
        [hw.astype(np.float64) * (1.0 - li[h]) for h in range(H)])
    wo_e = np.asarray(wo, np.float64) * colscale[None, :]

    base = (np.exp(np.sum(np.asarray(q1, np.float64) * np.asarray(k1, np.float64),
                          axis=-2))
            - np.exp(np.sum(np.asarray(q2, np.float64) * np.asarray(k2, np.float64),
                            axis=-2)))  # (H, 1)
    scale_h = -(H * base[:, 0] + li.sum())  # (H,)

    theta = 1.0 / (CONST ** (np.arange(0, DM, 2, dtype=np.float64) / DM))
    ang = np.arange(s, dtype=np.float64)[:, None] * theta[None, :]  # (s, 128)
    cost = np.cos(ang).T.astype(np.float16)  # (128, s)
    sint = np.sin(ang).T.astype(np.float16)

    x16 = x2.astype(np.float16)
    xtr = np.ascontiguousarray(x16.T)
    ph = _perm_core()
    in_maps = []
    for core in range(N_CORES):
        heads = range(core * HPC, (core + 1) * HPC)
        rows = core * HPC * DM + ph
        wqt = np.ascontiguousarray(wq_e[rows].T).astype(np.float16)
        wkt = np.ascontiguousarray(wk_e[rows].T).astype(np.float16)
        vrows = np.concatenate(
            [np.arange(h * HD, (h + 1) * HD) for h in heads])
        wvt = np.ascontiguousarray(wv_e[vrows].T).astype(np.float16)
        wot = np.ascontiguousarray(wo_e[:, vrows].T).astype(np.float16)
        lamc = scale_h[list(heads)].astype(np.float32).reshape(1, HPC)
        in_maps.append({
            "x": x16, "xtr": xtr, "wqt": wqt, "wkt": wkt, "wvt": wvt,
            "wot": wot, "cost": cost, "sint": sint, "lam": lamc,
        })
    return in_maps


_NC_CACHE = {}


def kernel(x, pre_norm_w, wq, wk, wv, wo, head_norm_w, q1, q2, k1, k2,
           lam_init):
    s = x.shape[1]
    if s not in _NC_CACHE:
        _NC_CACHE[s] = build_kernel(s)
    nc = _NC_CACHE[s]
    in_maps = prep_inputs(x, pre_norm_w, wq, wk, wv, wo, head_norm_w,
                          q1, q2, k1, k2, lam_init, s=s)
    res = run_bass_kernel_spmd(nc, in_maps, list(range(N_CORES)))
    acc = np.zeros((s, D), np.float64)
    for c in range(N_CORES):
        acc += res.results[c]["out"].astype(np.float64)
    out = acc.astype(np.float32) + np.asarray(x, np.float32).reshape(s, D)
    return out.reshape(1, s, D)


# revision 69
# speedup vs baseline: 1.0055x; 1.0055x over previous
"""Differential attention (dense_transformer) Trainium2 kernel.

Full-input contract: kernel(**inputs) takes the unsharded inputs of
reference.setup_inputs() and returns the full (1, S, D) float32 output.

Sharding: 16 heads across 8 cores (2 heads/core, tensor-parallel on the
q/k/v projection rows and wo columns). Each core computes a full (S, D)
partial of the output projection; the host sums partials and adds the
residual.
"""

import sys

for _p in ("/opt/trn_rl_repo", "/root/.axon_site/_ro/trn_rl_repo"):
    if _p not in sys.path:
        sys.path.insert(0, _p)

import math

import numpy as np

import concourse.bass as bass
import concourse.mybir as mybir
import concourse.tile as tile
from concourse import bacc
from concourse.bass import ts
from concourse.bass_utils import run_bass_kernel_spmd
from concourse.masks import make_identity, make_upper_triangular

F32 = mybir.dt.float32
F16 = mybir.dt.float16

# Problem constants
B, S, D = 1, 2048, 2048
H, C, HD = 16, 2, 128
DM = HD * C  # 256 per-head q/k dim
N_CORES = 8
HPC = H // N_CORES  # heads per core = 2
NHC = HPC * C  # head-comp blocks per core = 4
EPS = 1e-9
CONST = 10000.0
SQ = 512  # S_q super-tile width

# pool-size knobs (model-tuned)
CFG = {"ph1": 3, "pps": 5, "vpps": 3, "spp": 2, "ep": 10, "qkps": 2,
       "avps": 1, "ost": 3, "comb": 2, "attc": 2}


def build_kernel(s=S, loop_n=1):
    """Build the per-core Bass kernel (SPMD; per-core data differs).

    loop_n > 1 wraps the whole body in a hardware loop (timing only)."""
    import contextlib

    ns = s // 128  # S chunks of 128
    nj = s // SQ  # S_q super tiles
    kd = D // 128  # contraction chunks over D

    nc = bacc.Bacc("TRN2", target_bir_lowering=False, debug=False,
                   num_devices=N_CORES)

    x_d = nc.dram_tensor("x", [s, D], F16, kind="ExternalInput")
    xtr_d = nc.dram_tensor("xtr", [D, s], F16, kind="ExternalInput")
    wqt_d = nc.dram_tensor("wqt", [D, NHC * 128], F16, kind="ExternalInput")
    wkt_d = nc.dram_tensor("wkt", [D, NHC * 128], F16, kind="ExternalInput")
    wvt_d = nc.dram_tensor("wvt", [D, HPC * HD], F16, kind="ExternalInput")
    wot_d = nc.dram_tensor("wot", [HPC * HD, D], F16, kind="ExternalInput")
    cost_d = nc.dram_tensor("cost", [128, s], F16, kind="ExternalInput")
    sint_d = nc.dram_tensor("sint", [128, s], F16, kind="ExternalInput")
    lam_d = nc.dram_tensor("lam", [1, HPC], F32, kind="ExternalInput")
    out_d = nc.dram_tensor("out", [s, D], F16, kind="ExternalOutput")

    inv_sqrt_hd = 1.0 / math.sqrt(HD)
    I32 = mybir.dt.int32
    # float32 whose bit pattern is the rsqrt magic constant 0x5f3759df
    RSQRT_MAGIC = float(np.frombuffer(np.uint32(0x5F3759DF).tobytes(),
                                      np.float32)[0])

    def emit_rsqrt(out_f32, m_f32, ytile, ttile, ktile, shape, eng=None):
        """out = m^-0.5 via bit-trick seed + 2 Newton steps (no tables).
        ytile/ttile are f32 scratch APs of `shape`; ktile holds the magic."""
        eng = eng or nc.vector
        mul = mybir.AluOpType.mult
        eng.tensor_scalar(
            out=ytile.bitcast(I32), in0=m_f32.bitcast(I32), scalar1=1,
            scalar2=None, op0=mybir.AluOpType.logical_shift_right)
        eng.tensor_tensor(
            out=ytile.bitcast(I32), in0=ktile.bitcast(I32).to_broadcast(shape),
            in1=ytile.bitcast(I32), op=mybir.AluOpType.subtract)
        for it in range(2):
            tgt = out_f32 if it == 1 else ytile
            eng.tensor_tensor(out=ttile, in0=ytile, in1=ytile, op=mul)
            eng.tensor_tensor(out=ttile, in0=ttile, in1=m_f32, op=mul)
            eng.tensor_scalar(out=ttile, in0=ttile, scalar1=-0.5,
                              scalar2=1.5, op0=mul,
                              op1=mybir.AluOpType.add)
            eng.tensor_tensor(out=tgt, in0=ytile, in1=ttile, op=mul)

    with tile.TileContext(nc) as tc:
        with (
            (tc.For_i(0, loop_n, 1) if loop_n > 1
             else contextlib.nullcontext()),
            tc.tile_pool(name="const", bufs=1) as cp,
            tc.tile_pool(name="qk", bufs=1) as qkp,
            tc.tile_pool(name="vat", bufs=1) as vap,
        ):
            # ---- small persistent constants ----
            lam = cp.tile([128, HPC], F32, tag="lam")
            m0 = cp.tile([128, 128], F16, tag="m0")
            make_upper_triangular(nc, m0, val=1.0, diag=True)
            ident = cp.tile([128, 128], F16, tag="ident")
            make_identity(nc, ident)
            ktile = cp.tile([128, 1], F32, tag="ktile")
            nc.vector.memset(ktile, RSQRT_MAGIC)

            # persistent activations (split into dependency-granular tiles
            # so consumers start as soon as their slice is ready)
            qTs = [qkp.tile([128, s], F16, tag=f"qT{hc}", name=f"qT{hc}")
                   for hc in range(NHC)]
            kTs = [qkp.tile([128, s], F16, tag=f"kT{hc}", name=f"kT{hc}")
                   for hc in range(NHC)]
            # vaug per (head, S-quarter): [128, 4, 132]
            vaug = [[vap.tile([128, 4, 132], F16, tag=f"va{h}_{q}", name=f"va{h}_{q}")
                     for q in range(nj)] for h in range(HPC)]

            with (
                tc.tile_pool(name="wqkv", bufs=1) as wp,
                tc.tile_pool(name="ht", bufs=1) as htp,
            ):
                wqt = wp.tile([128, kd, NHC * 128], F16, tag="wqt")
                wkt = wp.tile([128, kd, NHC * 128], F16, tag="wkt")
                wvt = wp.tile([128, kd, HPC * HD], F16, tag="wvt")
                # hT split into S-quarters so projections of quarter j only
                # wait on that quarter's 4 transposes
                hts = [htp.tile([128, kd, SQ], F16, tag=f"ht{q}", name=f"ht{q}")
                       for q in range(nj)]

                # ---- phase 1: hT loads (host-pre-transposed x) + stats ----
                # The rmsnorm scale r is deferred off the critical path:
                # r multiplies q/k exactly via the (linear) RoPE tables and
                # v via its per-partition psum eviction. hT arrives as four
                # plain quarter loads of the host-transposed x; the natural
                # x tiles feed only the stats side-path.
                nc.sync.dma_start(out=wvt,
                                  in_=wvt_d.rearrange("(k p) m -> p k m", p=128))
                stats = wp.tile([128, ns], F32, tag="stats")
                rcol = wp.tile([128, ns], F32, tag="rcol")
                xtr_ap = xtr_d.rearrange("(k p) m -> p k m", p=128)
                with (
                    tc.tile_pool(name="ph1", bufs=4) as p1,
                    tc.tile_pool(name="ph1b", bufs=2) as p1b,
                    tc.tile_pool(name="rsc", bufs=2) as rscp,
                ):
                    def stat_group(g):
                        xts = []
                        for i in range(4 * g, 4 * g + 4):
                            xt = p1.tile([128, D], F16, tag="xt")
                            xts.append(xt)
                            nc.sync.dma_start(out=xt, in_=x_d[ts(i, 128), :])
                        for di, i in enumerate(range(4 * g, 4 * g + 4)):
                            sq = p1b.tile([128, D], F16, tag="sq")
                            nc.vector.scalar_tensor_tensor(
                                out=sq, in0=xts[di], scalar=1.0, in1=xts[di],
                                op0=mybir.AluOpType.mult,
                                op1=mybir.AluOpType.mult,
                                accum_out=stats[:, i:i + 1])
                        msl = stats[:, 4 * g:4 * g + 4]
                        nc.vector.tensor_scalar(
                            out=msl, in0=msl, scalar1=1.0 / D, scalar2=EPS,
                            op0=mybir.AluOpType.mult, op1=mybir.AluOpType.add)
                        ysc = rscp.tile([128, 4], F32, tag="ysc")
                        tsc = rscp.tile([128, 4], F32, tag="tsc")
                        emit_rsqrt(rcol[:, 4 * g:4 * g + 4], msl, ysc, tsc,
                                   ktile, (128, 4))

                    # interleave: each hT quarter unlocks projection work,
                    # its stats group unlocks the v evictions; q/k weight
                    # chunks slot in after the first/second quarter
                    for g in range(nj):
                        nc.sync.dma_start(out=hts[g],
                                          in_=xtr_ap[:, :, ts(g, SQ)])
                        stat_group(g)
                        if g == 0 or nj == 1:
                            for k in range(kd):
                                nc.sync.dma_start(out=wkt[:, k, :],
                                                  in_=wkt_d[ts(k, 128), :])
                        if g == 1 or nj == 1:
                            for k in range(kd):
                                nc.sync.dma_start(out=wqt[:, k, :],
                                                  in_=wqt_d[ts(k, 128), :])

                # ---- phase 2: projections + RoPE + repack ----
                # q/k are projected into a "split" row layout
                # [R0, R1, I0, I1] (R = rope-real rows, I = rope-imag rows;
                # j2 in {0,1} indexes the two 128-row groups of real parts).
                # RoPE then runs full-lane with partition-aligned operands,
                # and SBUF->SBUF DMAs repack into per-head-comp [xr;xi]
                # tiles (qT/kT) for K=128 attention matmuls.
                mul = mybir.AluOpType.mult
                with tc.tile_pool(name="pps", bufs=CFG["pps"], space="PSUM") as pps, \
                     tc.tile_pool(name="vpps", bufs=CFG["vpps"], space="PSUM") as vpps, \
                     tc.tile_pool(name="split", bufs=CFG["spp"]) as spp, \
                     tc.tile_pool(name="rope", bufs=1) as rp, \
                     tc.tile_pool(name="rdp", bufs=1, space="DRAM") as rdp, \
                     tc.tile_pool(name="ropec", bufs=1) as rcp:
                    cost = rcp.tile([128, s], F16, tag="cost")
                    nc.sync.dma_start(out=cost, in_=cost_d[:, :])
                    sint = rcp.tile([128, s], F16, tag="sint")
                    nc.sync.dma_start(out=sint, in_=sint_d[:, :])
                    # v first so attention's AV operands are ready early;
                    # the deferred rmsnorm scale rides the psum eviction
                    for i in range(ns):
                        ps = vpps.tile([128, HPC * HD], F32, tag="vps")
                        for k in range(kd):
                            nc.tensor.matmul(ps, hts[i // 4][:, k, ts(i % 4, 128)],
                                             wvt[:, k, :],
                                             start=(k == 0), stop=(k == kd - 1))
                        for h in range(HPC):
                            nc.vector.tensor_scalar_mul(
                                out=vaug[h][i // 4][:, i % 4, 0:128],
                                in0=ps[:, ts(h, 128)],
                                scalar1=rcol[:, i:i + 1])
                    for h in range(HPC):
                        for q in range(nj):
                            nc.vector.memset(vaug[h][q][:, :, 128:129], 1.0)
                    # fold r into the rope tables: cos/sin *= r[s] along the
                    # free axis (broadcast r via a DRAM bounce)
                    rc16g = rcp.tile([128, ns], F16, tag="rc16g")
                    nc.vector.tensor_copy(out=rc16g, in_=rcol)
                    rd = rdp.tile([1, s], F16, tag="rd")
                    nc.sync.dma_start(
                        out=rd[0:1, :].rearrange("o (i p) -> o p i", p=128),
                        in_=rc16g)
                    rbc = rcp.tile([128, s], F16, tag="rbc")
                    _rdap = rd[0:1, :]
                    nc.sync.dma_start(
                        out=rbc,
                        in_=bass.AP(tensor=_rdap.tensor, offset=_rdap.offset,
                                    ap=[[0, 128]] + list(_rdap.ap)[1:]))
                    nc.vector.tensor_tensor(out=cost, in0=cost, in1=rbc,
                                            op=mybir.AluOpType.mult)
                    nc.vector.tensor_tensor(out=sint, in0=sint, in1=rbc,
                                            op=mybir.AluOpType.mult)
                    # per (j2, tensor): project the (R_j2, I_j2) pair, RoPE,
                    # repack -- so head j2's attention can start while the
                    # other head is still projecting
                    for j2 in range(2):
                        for w_sb, t_sbs in ((wkt, kTs), (wqt, qTs)):
                            qs2 = spp.tile([128, 2, s], F16, tag="qs")
                            # quarter-major so each hT quarter unlocks both
                            # row-groups' matmuls as soon as it lands
                            for j in range(nj):
                                for mbi, mb in enumerate((j2, j2 + 2)):
                                    ps = pps.tile([128, SQ], F32, tag="ps")
                                    for k in range(kd):
                                        nc.tensor.matmul(
                                            ps, w_sb[:, k, ts(mb, 128)],
                                            hts[j][:, k, :],
                                            start=(k == 0), stop=(k == kd - 1))
                                    nc.scalar.activation(
                                        out=qs2[:, mbi, ts(j, SQ)], in_=ps,
                                        func=mybir.ActivationFunctionType.Copy)
                            # RoPE in place, full 128 lanes
                            xr = qs2[:, 0, :]
                            xi = qs2[:, 1, :]
                            t2 = rp.tile([128, s], F16, tag="t2")
                            t3 = rp.tile([128, s], F16, tag="t3")
                            nc.vector.tensor_tensor(out=t2, in0=xi, in1=sint,
                                                    op=mul)
                            nc.vector.tensor_tensor(out=t3, in0=xr, in1=sint,
                                                    op=mul)
                            nc.vector.tensor_tensor(out=xr, in0=xr, in1=cost,
                                                    op=mul)
                            nc.vector.tensor_tensor(out=xr, in0=xr, in1=t2,
                                                    op=mybir.AluOpType.subtract)
                            nc.vector.tensor_tensor(out=xi, in0=xi, in1=cost,
                                                    op=mul)
                            nc.vector.tensor_tensor(out=xi, in0=xi, in1=t3,
                                                    op=mybir.AluOpType.add)
                            # repack: hc tile = [xr(64) ; xi(64)]
                            for half in range(2):
                                hc = 2 * j2 + half
                                nc.sync.dma_start(
                                    out=t_sbs[hc][0:64, :],
                                    in_=qs2[ts(half, 64), 0, :])
                                nc.sync.dma_start(
                                    out=t_sbs[hc][64:128, :],
                                    in_=qs2[ts(half, 64), 1, :])

            # ---- phase 3 + 4: attention then output projection ----
            # attT per (head, S_q super-tile) so the output projection can
            # start on a row range as soon as both heads' combines finish.
            attT = [[qkp.tile([128, SQ], F16, tag=f"attT{h}_{q}", name=f"attT{h}_{q}")
                     for q in range(nj)] for h in range(HPC)]
            with (
                tc.tile_pool(name="ep", bufs=CFG["ep"]) as ep,
                tc.tile_pool(name="qkps", bufs=CFG["qkps"], space="PSUM") as qkps,
                tc.tile_pool(name="avps", bufs=CFG["avps"], space="PSUM") as avps,
                tc.tile_pool(name="tpps", bufs=1, space="PSUM") as tpps,
                tc.tile_pool(name="comb", bufs=CFG["comb"]) as cbp,
                tc.tile_pool(name="attc", bufs=CFG["attc"]) as atcp,
                tc.tile_pool(name="small", bufs=8) as smp,
                tc.tile_pool(name="wo", bufs=1) as wop,
                tc.tile_pool(name="ops", bufs=2, space="PSUM") as opsp,
                tc.tile_pool(name="ost", bufs=CFG["ost"]) as ostp,
            ):
                wot = wop.tile([128, HPC, D], F16, tag="wot")
                nc.sync.dma_start(out=wot,
                                  in_=wot_d.rearrange("(h p) n -> p h n", p=128))
                _lap = lam_d[:, :]
                nc.sync.dma_start(
                    out=lam,
                    in_=bass.AP(tensor=_lap.tensor, offset=_lap.offset,
                                ap=[[0, 128]] + list(_lap.ap)[1:]),
                )
                mul = mybir.AluOpType.mult
                add = mybir.AluOpType.add
                for j in range(nj - 1, -1, -1):
                    for head in range(HPC):
                        avsb = []
                        for c2 in range(C):
                            hc = C * head + c2
                            nblk = 4 * j + 4
                            # fused softmax normalization targets
                            attn_c = atcp.tile([128, 4, 128], F16,
                                               tag=f"attn{c2}", name=f"attn{c2}")
                            drc = smp.tile([128, 4, 1], F32, tag=f"drc{c2}",
                                           name=f"drc{c2}")
                            avsb.append(attn_c)
                            es = []
                            # S_k blocks in pairs: one 2-bank PSUM tile and
                            # (off-diagonal) one wide Exp per pair
                            for i2 in range(0, nblk, 2):
                                eps2 = qkps.tile([128, 2, SQ], F32, tag="eps")
                                et2 = ep.tile([128, 2, SQ], F16, tag="et")
                                diag = i2 >= 4 * j
                                for di in range(2):
                                    i = i2 + di
                                    c0 = 128 * max(i - 4 * j, 0)
                                    nc.tensor.matmul(
                                        eps2[:, di, c0:SQ], kTs[hc][:, ts(i, 128)],
                                        qTs[hc][:, SQ * j + c0:SQ * j + SQ],
                                        start=True, stop=True)
                                    if diag:
                                        nc.scalar.activation(
                                            out=et2[:, di, c0:SQ],
                                            in_=eps2[:, di, c0:SQ],
                                            func=mybir.ActivationFunctionType.Exp,
                                            scale=inv_sqrt_hd)
                                        nc.vector.tensor_tensor(
                                            out=et2[:, di, c0:c0 + 128],
                                            in0=et2[:, di, c0:c0 + 128],
                                            in1=m0, op=mul)
                                if not diag:
                                    nc.scalar.activation(
                                        out=et2, in_=eps2,
                                        func=mybir.ActivationFunctionType.Exp,
                                        scale=inv_sqrt_hd)
                                es.append(et2)
                            for m in range(4):
                                avm = avps.tile([128, 129], F32, tag="avm")
                                for i in range(4 * j + m + 1):
                                    nc.tensor.matmul(
                                        avm, es[i // 2][:, i % 2, ts(m, 128)],
                                        vaug[head][i // 4][:, i % 4, 0:129],
                                        start=(i == 0), stop=(i == 4 * j + m))
                                nc.vector.reciprocal(out=drc[:, m, :],
                                                     in_=avm[:, 128:129])
                                nc.vector.tensor_scalar_mul(
                                    out=attn_c[:, m, :], in0=avm[:, 0:128],
                                    scalar1=drc[:, m, :])
                        # combine components + head RMSNorm (f16, 2x mode)
                        comb = cbp.tile([128, 4, 128], F16, tag="comb")
                        nc.vector.scalar_tensor_tensor(
                            out=comb, in0=avsb[1], scalar=lam[:, head:head + 1],
                            in1=avsb[0], op0=mul, op1=add)
                        tt = cbp.tile([128, 4, 128], F16, tag="tt")
                        nc.vector.tensor_tensor(out=tt, in0=comb, in1=comb, op=mul)
                        ssum = smp.tile([128, 4, 1], F32, tag="ssum")
                        nc.vector.reduce_sum(out=ssum, in_=tt,
                                             axis=mybir.AxisListType.X)
                        nc.vector.tensor_scalar(
                            out=ssum, in0=ssum, scalar1=1.0 / HD, scalar2=EPS,
                            op0=mul, op1=add)
                        rf = smp.tile([128, 4, 1], F32, tag="rf")
                        ycb = smp.tile([128, 4, 1], F32, tag="ycb")
                        tcb = smp.tile([128, 4, 1], F32, tag="tcb")
                        emit_rsqrt(rf, ssum, ycb, tcb,
                                   ktile[:, :, None], (128, 4, 1))
                        a16 = cbp.tile([128, 4, 128], F16, tag="a16")
                        nc.vector.tensor_tensor(
                            out=a16, in0=comb, in1=rf.to_broadcast((128, 4, 128)),
                            op=mul)
                        for mm in range(4):
                            tp = tpps.tile([128, 128], F16, tag="tp")
                            nc.tensor.transpose(tp, a16[:, mm, :], ident)
                            nc.vector.tensor_copy(
                                out=attT[head][j][:, ts(mm, 128)], in_=tp)
                        if head == HPC - 1:
                            # output projection for this super-tile's rows
                            for sm in range(4 * j, 4 * j + 4):
                                for dn in range(D // SQ):
                                    ps = opsp.tile([128, SQ], F32, tag="ops")
                                    for h in range(HPC):
                                        nc.tensor.matmul(
                                            ps,
                                            attT[h][sm // 4][:, ts(sm % 4, 128)],
                                            wot[:, h, ts(dn, SQ)],
                                            start=(h == 0), stop=(h == HPC - 1))
                                    ost = ostp.tile([128, SQ], F16, tag="ost")
                                    if j <= 1:
                                        nc.scalar.activation(
                                            out=ost, in_=ps,
                                            func=mybir.ActivationFunctionType.Copy)
                                    else:
                                        nc.vector.tensor_copy(out=ost, in_=ps)
                                    nc.sync.dma_start(
                                        out=out_d[ts(sm, 128), ts(dn, SQ)],
                                        in_=ost)

    nc.compile()
    return nc


def _perm_core():
    """Row permutation of one core's HPC*DM q/k rows into the split layout
    [R0..R_{HPC-1}, I0..I_{HPC-1}]: R_h = rope-real (even) rows of head h for
    both components, I_h = rope-imag (odd) rows. Within each 128-row block,
    rows follow theta-pair order 0..127."""
    evens = [h * DM + 128 * c + 2 * t
             for h in range(HPC) for c in range(C) for t in range(64)]
    odds = [h * DM + 128 * c + 2 * t + 1
            for h in range(HPC) for c in range(C) for t in range(64)]
    return np.array(evens + odds)


def prep_inputs(x, pre_norm_w, wq, wk, wv, wo, head_norm_w, q1, q2, k1, k2,
                lam_init, s=S):
    """Host-side prep: fold norms/lambdas into weights, permute q/k rows,
    transpose, cast fp16, build rope tables; returns per-core input maps."""
    x2 = np.asarray(x, np.float32).reshape(s, D)
    pw = np.asarray(pre_norm_w, np.float32)
    hw = np.asarray(head_norm_w, np.float32)
    li = np.asarray(lam_init, np.float64)

    wq_e = (np.asarray(wq, np.float64) * pw[None, :])
    wk_e = (np.asarray(wk, np.float64) * pw[None, :])
    wv_e = (np.asarray(wv, np.float64) * pw[None, :])
    # wo: out = att_normed * (1-lam) @ wo.T ; head_norm_w folds per att dim
    colscale = np.concatenate(
        [hw.astype(np.float64) * (1.0 - li[h]) for h in range(H)])
    wo_e = np.asarray(wo, np.float64) * colscale[None, :]

    base = (np.exp(np.sum(np.asarray(q1, np.float64) * np.asarray(k1, np.float64),
                          axis=-2))
            - np.exp(np.sum(np.asarray(q2, np.float64) * np.asarray(k2, np.float64),
                            axis=-2)))  # (H, 1)
    scale_h = -(H * base[:, 0] + li.sum())  # (H,)

    theta = 1.0 / (CONST ** (np.arange(0, DM, 2, dtype=np.float64) / DM))
    ang = np.arange(s, dtype=np.float64)[:, None] * theta[None, :]  # (s, 128)
    cost = np.cos(ang).T.astype(np.float16)  # (128, s)
    sint = np.sin(ang).T.astype(np.float16)

    x16 = x2.astype(np.float16)
    xtr = np.ascontiguousarray(x16.T)
    ph = _perm_core()
    in_maps = []
    for core in range(N_CORES):
        heads = range(core * HPC, (core + 1) * HPC)
        rows = core * HPC * DM + ph
        wqt = np.ascontiguousarray(wq_e[rows].T).astype(np.float16)
        wkt = np.ascontiguousarray(wk_e[rows].T).astype(np.float16)
        vrows = np.concatenate(
            [np.arange(h * HD, (h + 1) * HD) for h in heads])
        wvt = np.ascontiguousarray(wv_e[vrows].T).astype(np.float16)
        wot = np.ascontiguousarray(wo_e[:, vrows].T).astype(np.float16)
        lamc = scale_h[list(heads)].astype(np.float32).reshape(1, HPC)
        in_maps.append({
            "x": x16, "xtr": xtr, "wqt": wqt, "wkt": wkt, "wvt": wvt,
            "wot": wot, "cost": cost, "sint": sint, "lam": lamc,
        })
    return in_maps


_NC_CACHE = {}


def kernel(x, pre_norm_w, wq, wk, wv, wo, head_norm_w, q1, q2, k1, k2,
           lam_init):
    s = x.shape[1]
    if s not in _NC_CACHE:
        _NC_CACHE[s] = build_kernel(s)
    nc = _NC_CACHE[s]
    in_maps = prep_inputs(x, pre_norm_w, wq, wk, wv, wo, head_norm_w,
                          q1, q2, k1, k2, lam_init, s=s)
    res = run_bass_kernel_spmd(nc, in_maps, list(range(N_CORES)))
    acc = np.zeros((s, D), np.float64)
    for c in range(N_CORES):
        acc += res.results[c]["out"].astype(np.float64)
    out = acc.astype(np.float32) + np.asarray(x, np.float32).reshape(s, D)
    return out.reshape(1, s, D)


# revision 71
# speedup vs baseline: 1.0507x; 1.0450x over previous
"""Differential attention (dense_transformer) Trainium2 kernel.

Full-input contract: kernel(**inputs) takes the unsharded inputs of
reference.setup_inputs() and returns the full (1, S, D) float32 output.

Sharding: 16 heads across 8 cores (2 heads/core, tensor-parallel on the
q/k/v projection rows and wo columns). Each core computes a full (S, D)
partial of the output projection; the host sums partials and adds the
residual.
"""

import sys

for _p in ("/opt/trn_rl_repo", "/root/.axon_site/_ro/trn_rl_repo"):
    if _p not in sys.path:
        sys.path.insert(0, _p)

import math

import numpy as np

import concourse.bass as bass
import concourse.mybir as mybir
import concourse.tile as tile
from concourse import bacc
from concourse.bass import ts
from concourse.bass_utils import run_bass_kernel_spmd
from concourse.masks import make_identity, make_upper_triangular

F32 = mybir.dt.float32
F16 = mybir.dt.float16

# Problem constants
B, S, D = 1, 2048, 2048
H, C, HD = 16, 2, 128
DM = HD * C  # 256 per-head q/k dim
N_CORES = 8
HPC = H // N_CORES  # heads per core = 2
NHC = HPC * C  # head-comp blocks per core = 4
EPS = 1e-9
CONST = 10000.0
SQ = 512  # S_q super-tile width

# pool-size knobs (model-tuned)
CFG = {"ph1": 3, "pps": 5, "vpps": 3, "spp": 2, "ep": 10, "qkps": 2,
       "avps": 1, "ost": 6, "comb": 2, "attc": 2}


def build_kernel(s=S, loop_n=1):
    """Build the per-core Bass kernel (SPMD; per-core data differs).

    loop_n > 1 wraps the whole body in a hardware loop (timing only)."""
    import contextlib

    ns = s // 128  # S chunks of 128
    nj = s // SQ  # S_q super tiles
    kd = D // 128  # contraction chunks over D

    nc = bacc.Bacc("TRN2", target_bir_lowering=False, debug=False,
                   num_devices=N_CORES)

    x_d = nc.dram_tensor("x", [s, D], F16, kind="ExternalInput")
    xtr_d = nc.dram_tensor("xtr", [D, s], F16, kind="ExternalInput")
    wqt_d = nc.dram_tensor("wqt", [D, NHC * 128], F16, kind="ExternalInput")
    wkt_d = nc.dram_tensor("wkt", [D, NHC * 128], F16, kind="ExternalInput")
    wvt_d = nc.dram_tensor("wvt", [D, HPC * HD], F16, kind="ExternalInput")
    wot_d = nc.dram_tensor("wot", [HPC * HD, D], F16, kind="ExternalInput")
    cost_d = nc.dram_tensor("cost", [128, s], F16, kind="ExternalInput")
    sint_d = nc.dram_tensor("sint", [128, s], F16, kind="ExternalInput")
    lam_d = nc.dram_tensor("lam", [1, HPC], F32, kind="ExternalInput")
    out_d = nc.dram_tensor("out", [s, D], F16, kind="ExternalOutput")

    inv_sqrt_hd = 1.0 / math.sqrt(HD)
    I32 = mybir.dt.int32
    # float32 whose bit pattern is the rsqrt magic constant 0x5f3759df
    RSQRT_MAGIC = float(np.frombuffer(np.uint32(0x5F3759DF).tobytes(),
                                      np.float32)[0])

    def emit_rsqrt(out_f32, m_f32, ytile, ttile, ktile, shape, eng=None):
        """out = m^-0.5 via bit-trick seed + 2 Newton steps (no tables).
        ytile/ttile are f32 scratch APs of `shape`; ktile holds the magic."""
        eng = eng or nc.vector
        mul = mybir.AluOpType.mult
        eng.tensor_scalar(
            out=ytile.bitcast(I32), in0=m_f32.bitcast(I32), scalar1=1,
            scalar2=None, op0=mybir.AluOpType.logical_shift_right)
        eng.tensor_tensor(
            out=ytile.bitcast(I32), in0=ktile.bitcast(I32).to_broadcast(shape),
            in1=ytile.bitcast(I32), op=mybir.AluOpType.subtract)
        for it in range(2):
            tgt = out_f32 if it == 1 else ytile
            eng.tensor_tensor(out=ttile, in0=ytile, in1=ytile, op=mul)
            eng.tensor_tensor(out=ttile, in0=ttile, in1=m_f32, op=mul)
            eng.tensor_scalar(out=ttile, in0=ttile, scalar1=-0.5,
                              scalar2=1.5, op0=mul,
                              op1=mybir.AluOpType.add)
            eng.tensor_tensor(out=tgt, in0=ytile, in1=ttile, op=mul)

    with tile.TileContext(nc) as tc:
        with (
            (tc.For_i(0, loop_n, 1) if loop_n > 1
             else contextlib.nullcontext()),
            tc.tile_pool(name="const", bufs=1) as cp,
            tc.tile_pool(name="qk", bufs=1) as qkp,
            tc.tile_pool(name="vat", bufs=1) as vap,
        ):
            # ---- small persistent constants ----
            lam = cp.tile([128, HPC], F32, tag="lam")
            m0 = cp.tile([128, 128], F16, tag="m0")
            make_upper_triangular(nc, m0, val=1.0, diag=True)
            ident = cp.tile([128, 128], F16, tag="ident")
            make_identity(nc, ident)
            ktile = cp.tile([128, 1], F32, tag="ktile")
            nc.vector.memset(ktile, RSQRT_MAGIC)

            # persistent activations (split into dependency-granular tiles
            # so consumers start as soon as their slice is ready)
            qTs = [qkp.tile([128, s], F16, tag=f"qT{hc}", name=f"qT{hc}")
                   for hc in range(NHC)]
            kTs = [qkp.tile([128, s], F16, tag=f"kT{hc}", name=f"kT{hc}")
                   for hc in range(NHC)]
            # vaug per (head, S-quarter): [128, 4, 132]
            vaug = [[vap.tile([128, 4, 132], F16, tag=f"va{h}_{q}", name=f"va{h}_{q}")
                     for q in range(nj)] for h in range(HPC)]

            with (
                tc.tile_pool(name="wqkv", bufs=1) as wp,
                tc.tile_pool(name="ht", bufs=1) as htp,
            ):
                wqt = wp.tile([128, kd, NHC * 128], F16, tag="wqt")
                wkt = wp.tile([128, kd, NHC * 128], F16, tag="wkt")
                wvt = wp.tile([128, kd, HPC * HD], F16, tag="wvt")
                # hT split into S-quarters so projections of quarter j only
                # wait on that quarter's 4 transposes
                hts = [htp.tile([128, kd, SQ], F16, tag=f"ht{q}", name=f"ht{q}")
                       for q in range(nj)]

                # ---- phase 1: hT loads (host-pre-transposed x) + stats ----
                # The rmsnorm scale r is deferred off the critical path:
                # r multiplies q/k exactly via the (linear) RoPE tables and
                # v via its per-partition psum eviction. hT arrives as four
                # plain quarter loads of the host-transposed x; the natural
                # x tiles feed only the stats side-path.
                nc.sync.dma_start(out=wvt,
                                  in_=wvt_d.rearrange("(k p) m -> p k m", p=128))
                stats = wp.tile([128, ns], F32, tag="stats")
                rcol = wp.tile([128, ns], F32, tag="rcol")
                xtr_ap = xtr_d.rearrange("(k p) m -> p k m", p=128)
                with (
                    tc.tile_pool(name="ph1", bufs=4) as p1,
                    tc.tile_pool(name="ph1b", bufs=2) as p1b,
                    tc.tile_pool(name="rsc", bufs=2) as rscp,
                ):
                    def stat_group(g):
                        xts = []
                        for i in range(4 * g, 4 * g + 4):
                            xt = p1.tile([128, D], F16, tag="xt")
                            xts.append(xt)
                            nc.sync.dma_start(out=xt, in_=x_d[ts(i, 128), :])
                        for di, i in enumerate(range(4 * g, 4 * g + 4)):
                            sq = p1b.tile([128, D], F16, tag="sq")
                            nc.scalar.activation(
                                out=sq, in_=xts[di],
                                func=mybir.ActivationFunctionType.Square,
                                accum_out=stats[:, i:i + 1])
                        msl = stats[:, 4 * g:4 * g + 4]
                        nc.vector.tensor_scalar(
                            out=msl, in0=msl, scalar1=1.0 / D, scalar2=EPS,
                            op0=mybir.AluOpType.mult, op1=mybir.AluOpType.add)
                        ysc = rscp.tile([128, 4], F32, tag="ysc")
                        tsc = rscp.tile([128, 4], F32, tag="tsc")
                        emit_rsqrt(rcol[:, 4 * g:4 * g + 4], msl, ysc, tsc,
                                   ktile, (128, 4))

                    # interleave: each hT quarter unlocks projection work,
                    # its stats group unlocks the v evictions; q/k weight
                    # chunks slot in after the first/second quarter
                    for g in range(nj):
                        nc.sync.dma_start(out=hts[g],
                                          in_=xtr_ap[:, :, ts(g, SQ)])
                        stat_group(g)
                        if g == 0 or nj == 1:
                            for k in range(kd):
                                nc.sync.dma_start(out=wkt[:, k, :],
                                                  in_=wkt_d[ts(k, 128), :])
                        if g == 1 or nj == 1:
                            for k in range(kd):
                                nc.sync.dma_start(out=wqt[:, k, :],
                                                  in_=wqt_d[ts(k, 128), :])

                # ---- phase 2: projections + RoPE + repack ----
                # q/k are projected into a "split" row layout
                # [R0, R1, I0, I1] (R = rope-real rows, I = rope-imag rows;
                # j2 in {0,1} indexes the two 128-row groups of real parts).
                # RoPE then runs full-lane with partition-aligned operands,
                # and SBUF->SBUF DMAs repack into per-head-comp [xr;xi]
                # tiles (qT/kT) for K=128 attention matmuls.
                mul = mybir.AluOpType.mult
                with tc.tile_pool(name="pps", bufs=CFG["pps"], space="PSUM") as pps, \
                     tc.tile_pool(name="vpps", bufs=CFG["vpps"], space="PSUM") as vpps, \
                     tc.tile_pool(name="split", bufs=CFG["spp"]) as spp, \
                     tc.tile_pool(name="rope", bufs=1) as rp, \
                     tc.tile_pool(name="rdp", bufs=1, space="DRAM") as rdp, \
                     tc.tile_pool(name="ropec", bufs=1) as rcp:
                    cost = rcp.tile([128, s], F16, tag="cost")
                    nc.sync.dma_start(out=cost, in_=cost_d[:, :])
                    sint = rcp.tile([128, s], F16, tag="sint")
                    nc.sync.dma_start(out=sint, in_=sint_d[:, :])
                    # v first so attention's AV operands are ready early;
                    # the deferred rmsnorm scale rides the psum eviction
                    for i in range(ns):
                        ps = vpps.tile([128, HPC * HD], F32, tag="vps")
                        for k in range(kd):
                            nc.tensor.matmul(ps, hts[i // 4][:, k, ts(i % 4, 128)],
                                             wvt[:, k, :],
                                             start=(k == 0), stop=(k == kd - 1))
                        for h in range(HPC):
                            nc.vector.tensor_scalar_mul(
                                out=vaug[h][i // 4][:, i % 4, 0:128],
                                in0=ps[:, ts(h, 128)],
                                scalar1=rcol[:, i:i + 1])
                    for h in range(HPC):
                        for q in range(nj):
                            nc.vector.memset(vaug[h][q][:, :, 128:129], 1.0)
                    # fold r into the rope tables: cos/sin *= r[s] along the
                    # free axis (broadcast r via a DRAM bounce)
                    rc16g = rcp.tile([128, ns], F16, tag="rc16g")
                    nc.vector.tensor_copy(out=rc16g, in_=rcol)
                    rd = rdp.tile([1, s], F16, tag="rd")
                    nc.sync.dma_start(
                        out=rd[0:1, :].rearrange("o (i p) -> o p i", p=128),
                        in_=rc16g)
                    rbc = rcp.tile([128, s], F16, tag="rbc")
                    _rdap = rd[0:1, :]
                    nc.sync.dma_start(
                        out=rbc,
                        in_=bass.AP(tensor=_rdap.tensor, offset=_rdap.offset,
                                    ap=[[0, 128]] + list(_rdap.ap)[1:]))
                    nc.vector.tensor_tensor(out=cost, in0=cost, in1=rbc,
                                            op=mybir.AluOpType.mult)
                    nc.vector.tensor_tensor(out=sint, in0=sint, in1=rbc,
                                            op=mybir.AluOpType.mult)
                    # per (j2, tensor): project the (R_j2, I_j2) pair, RoPE,
                    # repack -- so head j2's attention can start while the
                    # other head is still projecting
                    for j2 in range(2):
                        for w_sb, t_sbs in ((wkt, kTs), (wqt, qTs)):
                            qs2 = spp.tile([128, 2, s], F16, tag="qs")
                            # quarter-major so each hT quarter unlocks both
                            # row-groups' matmuls as soon as it lands
                            for j in range(nj):
                                for mbi, mb in enumerate((j2, j2 + 2)):
                                    ps = pps.tile([128, SQ], F32, tag="ps")
                                    for k in range(kd):
                                        nc.tensor.matmul(
                                            ps, w_sb[:, k, ts(mb, 128)],
                                            hts[j][:, k, :],
                                            start=(k == 0), stop=(k == kd - 1))
                                    nc.scalar.activation(
                                        out=qs2[:, mbi, ts(j, SQ)], in_=ps,
                                        func=mybir.ActivationFunctionType.Copy)
                            # RoPE in place, full 128 lanes
                            xr = qs2[:, 0, :]
                            xi = qs2[:, 1, :]
                            t2 = rp.tile([128, s], F16, tag="t2")
                            t3 = rp.tile([128, s], F16, tag="t3")
                            nc.vector.tensor_tensor(out=t2, in0=xi, in1=sint,
                                                    op=mul)
                            nc.vector.tensor_tensor(out=t3, in0=xr, in1=sint,
                                                    op=mul)
                            nc.vector.tensor_tensor(out=xr, in0=xr, in1=cost,
                                                    op=mul)
                            nc.vector.tensor_tensor(out=xr, in0=xr, in1=t2,
                                                    op=mybir.AluOpType.subtract)
                            nc.vector.tensor_tensor(out=xi, in0=xi, in1=cost,
                                                    op=mul)
                            nc.vector.tensor_tensor(out=xi, in0=xi, in1=t3,
                                                    op=mybir.AluOpType.add)
                            # repack: hc tile = [xr(64) ; xi(64)]
                            for half in range(2):
                                hc = 2 * j2 + half
                                nc.sync.dma_start(
                                    out=t_sbs[hc][0:64, :],
                                    in_=qs2[ts(half, 64), 0, :])
                                nc.sync.dma_start(
                                    out=t_sbs[hc][64:128, :],
                                    in_=qs2[ts(half, 64), 1, :])

            # ---- phase 3 + 4: attention then output projection ----
            # attT per (head, S_q super-tile) so the output projection can
            # start on a row range as soon as both heads' combines finish.
            attT = [[qkp.tile([128, SQ], F16, tag=f"attT{h}_{q}", name=f"attT{h}_{q}")
                     for q in range(nj)] for h in range(HPC)]
            with (
                tc.tile_pool(name="ep", bufs=CFG["ep"]) as ep,
                tc.tile_pool(name="qkps", bufs=CFG["qkps"], space="PSUM") as qkps,
                tc.tile_pool(name="avps", bufs=CFG["avps"], space="PSUM") as avps,
                tc.tile_pool(name="tpps", bufs=1, space="PSUM") as tpps,
                tc.tile_pool(name="comb", bufs=CFG["comb"]) as cbp,
                tc.tile_pool(name="attc", bufs=CFG["attc"]) as atcp,
                tc.tile_pool(name="small", bufs=8) as smp,
                tc.tile_pool(name="wo", bufs=1) as wop,
                tc.tile_pool(name="ops", bufs=2, space="PSUM") as opsp,
                tc.tile_pool(name="ost", bufs=CFG["ost"]) as ostp,
            ):
                wot = wop.tile([128, HPC, D], F16, tag="wot")
                nc.sync.dma_start(out=wot,
                                  in_=wot_d.rearrange("(h p) n -> p h n", p=128))
                _lap = lam_d[:, :]
                nc.sync.dma_start(
                    out=lam,
                    in_=bass.AP(tensor=_lap.tensor, offset=_lap.offset,
                                ap=[[0, 128]] + list(_lap.ap)[1:]),
                )
                mul = mybir.AluOpType.mult
                add = mybir.AluOpType.add
                for j in range(nj - 1, -1, -1):
                    for head in range(HPC):
                        avsb = []
                        for c2 in range(C):
                            hc = C * head + c2
                            nblk = 4 * j + 4
                            # fused softmax normalization targets
                            attn_c = atcp.tile([128, 4, 128], F16,
                                               tag=f"attn{c2}", name=f"attn{c2}")
                            drc = smp.tile([128, 4, 1], F32, tag=f"drc{c2}",
                                           name=f"drc{c2}")
                            avsb.append(attn_c)
                            es = []
                            # S_k blocks in pairs: one 2-bank PSUM tile and
                            # (off-diagonal) one wide Exp per pair
                            for i2 in range(0, nblk, 2):
                                eps2 = qkps.tile([128, 2, SQ], F32, tag="eps")
                                et2 = ep.tile([128, 2, SQ], F16, tag="et")
                                diag = i2 >= 4 * j
                                for di in range(2):
                                    i = i2 + di
                                    c0 = 128 * max(i - 4 * j, 0)
                                    nc.tensor.matmul(
                                        eps2[:, di, c0:SQ], kTs[hc][:, ts(i, 128)],
                                        qTs[hc][:, SQ * j + c0:SQ * j + SQ],
                                        start=True, stop=True)
                                    if diag:
                                        nc.scalar.activation(
                                            out=et2[:, di, c0:SQ],
                                            in_=eps2[:, di, c0:SQ],
                                            func=mybir.ActivationFunctionType.Exp,
                                            scale=inv_sqrt_hd)
                                        nc.vector.tensor_tensor(
                                            out=et2[:, di, c0:c0 + 128],
                                            in0=et2[:, di, c0:c0 + 128],
                                            in1=m0, op=mul)
                                if not diag:
                                    nc.scalar.activation(
                                        out=et2, in_=eps2,
                                        func=mybir.ActivationFunctionType.Exp,
                                        scale=inv_sqrt_hd)
                                es.append(et2)
                            for m in range(4):
                                avm = avps.tile([128, 129], F32, tag="avm")
                                for i in range(4 * j + m + 1):
                                    nc.tensor.matmul(
                                        avm, es[i // 2][:, i % 2, ts(m, 128)],
                                        vaug[head][i // 4][:, i % 4, 0:129],
                                        start=(i == 0), stop=(i == 4 * j + m))
                                nc.vector.reciprocal(out=drc[:, m, :],
                                                     in_=avm[:, 128:129])
                                nc.vector.tensor_scalar_mul(
                                    out=attn_c[:, m, :], in0=avm[:, 0:128],
                                    scalar1=drc[:, m, :])
                        # combine components + head RMSNorm (f16, 2x mode)
                        comb = cbp.tile([128, 4, 128], F16, tag="comb")
                        nc.vector.scalar_tensor_tensor(
                            out=comb, in0=avsb[1], scalar=lam[:, head:head + 1],
                            in1=avsb[0], op0=mul, op1=add)
                        tt = cbp.tile([128, 4, 128], F16, tag="tt")
                        nc.vector.tensor_tensor(out=tt, in0=comb, in1=comb, op=mul)
                        ssum = smp.tile([128, 4, 1], F32, tag="ssum")
                        nc.vector.reduce_sum(out=ssum, in_=tt,
                                             axis=mybir.AxisListType.X)
                        nc.vector.tensor_scalar(
                            out=ssum, in0=ssum, scalar1=1.0 / HD, scalar2=EPS,
                            op0=mul, op1=add)
                        rf = smp.tile([128, 4, 1], F32, tag="rf")
                        ycb = smp.tile([128, 4, 1], F32, tag="ycb")
                        tcb = smp.tile([128, 4, 1], F32, tag="tcb")
                        emit_rsqrt(rf, ssum, ycb, tcb,
                                   ktile[:, :, None], (128, 4, 1))
                        a16 = cbp.tile([128, 4, 128], F16, tag="a16")
                        nc.vector.tensor_tensor(
                            out=a16, in0=comb, in1=rf.to_broadcast((128, 4, 128)),
                            op=mul)
                        for mm in range(4):
                            tp = tpps.tile([128, 128], F16, tag="tp")
                            nc.tensor.transpose(tp, a16[:, mm, :], ident)
                            nc.vector.tensor_copy(
                                out=attT[head][j][:, ts(mm, 128)], in_=tp)
                        if head == HPC - 1:
                            # output projection for this super-tile's rows
                            for sm in range(4 * j, 4 * j + 4):
                                for dn in range(D // SQ):
                                    ps = opsp.tile([128, SQ], F32, tag="ops")
                                    for h in range(HPC):
                                        nc.tensor.matmul(
                                            ps,
                                            attT[h][sm // 4][:, ts(sm % 4, 128)],
                                            wot[:, h, ts(dn, SQ)],
                                            start=(h == 0), stop=(h == HPC - 1))
                                    ost = ostp.tile([128, SQ], F16, tag="ost")
                                    if j <= 1:
                                        nc.scalar.activation(
                                            out=ost, in_=ps,
                                            func=mybir.ActivationFunctionType.Copy)
                                    else:
                                        nc.vector.tensor_copy(out=ost, in_=ps)
                                    nc.sync.dma_start(
                                        out=out_d[ts(sm, 128), ts(dn, SQ)],
                                        in_=ost)

    nc.compile()
    return nc


def _perm_core():
    """Row permutation of one core's HPC*DM q/k rows into the split layout
    [R0..R_{HPC-1}, I0..I_{HPC-1}]: R_h = rope-real (even) rows of head h for
    both components, I_h = rope-imag (odd) rows. Within each 128-row block,
    rows follow theta-pair order 0..127."""
    evens = [h * DM + 128 * c + 2 * t
             for h in range(HPC) for c in range(C) for t in range(64)]
    odds = [h * DM + 128 * c + 2 * t + 1
            for h in range(HPC) for c in range(C) for t in range(64)]
    return np.array(evens + odds)


def prep_inputs(x, pre_norm_w, wq, wk, wv, wo, head_norm_w, q1, q2, k1, k2,
                lam_init, s=S):
    """Host-side prep: fold norms/lambdas into weights, permute q/k rows,
    transpose, cast fp16, build rope tables; returns per-core input maps."""
    x2 = np.asarray(x, np.float32).reshape(s, D)
    pw = np.asarray(pre_norm_w, np.float32)
    hw = np.asarray(head_norm_w, np.float32)
    li = np.asarray(lam_init, np.float64)

    wq_e = (np.asarray(wq, np.float64) * pw[None, :])
    wk_e = (np.asarray(wk, np.float64) * pw[None, :])
    wv_e = (np.asarray(wv, np.float64) * pw[None, :])
    # wo: out = att_normed * (1-lam) @ wo.T ; head_norm_w folds per att dim
    colscale = np.concatenate(
        [hw.astype(np.float64) * (1.0 - li[h]) for h in range(H)])
    wo_e = np.asarray(wo, np.float64) * colscale[None, :]

    base = (np.exp(np.sum(np.asarray(q1, np.float64) * np.asarray(k1, np.float64),
                          axis=-2))
            - np.exp(np.sum(np.asarray(q2, np.float64) * np.asarray(k2, np.float64),
                            axis=-2)))  # (H, 1)
    scale_h = -(H * base[:, 0] + li.sum())  # (H,)

    theta = 1.0 / (CONST ** (np.arange(0, DM, 2, dtype=np.float64) / DM))
    ang = np.arange(s, dtype=np.float64)[:, None] * theta[None, :]  # (s, 128)
    cost = np.cos(ang).T.astype(np.float16)  # (128, s)
    sint = np.sin(ang).T.astype(np.float16)

    x16 = x2.astype(np.float16)
    xtr = np.ascontiguousarray(x16.T)
    ph = _perm_core()
    in_maps = []
    for core in range(N_CORES):
        heads = range(core * HPC, (core + 1) * HPC)
        rows = core * HPC * DM + ph
        wqt = np.ascontiguousarray(wq_e[rows].T).astype(np.float16)
        wkt = np.ascontiguousarray(wk_e[rows].T).astype(np.float16)
        vrows = np.concatenate(
            [np.arange(h * HD, (h + 1) * HD) for h in heads])
        wvt = np.ascontiguousarray(wv_e[vrows].T).astype(np.float16)
        wot = np.ascontiguousarray(wo_e[:, vrows].T).astype(np.float16)
        lamc = scale_h[list(heads)].astype(np.float32).reshape(1, HPC)
        in_maps.append({
            "x": x16, "xtr": xtr, "wqt": wqt, "wkt": wkt, "wvt": wvt,
            "wot": wot, "cost": cost, "sint": sint, "lam": lamc,
        })
    return in_maps


_NC_CACHE = {}


def kernel(x, pre_norm_w, wq, wk, wv, wo, head_norm_w, q1, q2, k1, k2,
           lam_init):
    s = x.shape[1]
    if s not in _NC_CACHE:
        _NC_CACHE[s] = build_kernel(s)
    nc = _NC_CACHE[s]
    in_maps = prep_inputs(x, pre_norm_w, wq, wk, wv, wo, head_norm_w,
                          q1, q2, k1, k2, lam_init, s=s)
    res = run_bass_kernel_spmd(nc, in_maps, list(range(N_CORES)))
    acc = np.zeros((s, D), np.float64)
    for c in range(N_CORES):
        acc += res.results[c]["out"].astype(np.float64)
    out = acc.astype(np.float32) + np.asarray(x, np.float32).reshape(s, D)
    return out.reshape(1, s, D)


# revision 72
# speedup vs baseline: 1.0612x; 1.0100x over previous
"""Differential attention (dense_transformer) Trainium2 kernel.

Full-input contract: kernel(**inputs) takes the unsharded inputs of
reference.setup_inputs() and returns the full (1, S, D) float32 output.

Sharding: 16 heads across 8 cores (2 heads/core, tensor-parallel on the
q/k/v projection rows and wo columns). Each core computes a full (S, D)
partial of the output projection; the host sums partials and adds the
residual.
"""

import sys

for _p in ("/opt/trn_rl_repo", "/root/.axon_site/_ro/trn_rl_repo"):
    if _p not in sys.path:
        sys.path.insert(0, _p)

import math

import numpy as np

import concourse.bass as bass
import concourse.mybir as mybir
import concourse.tile as tile
from concourse import bacc
from concourse.bass import ts
from concourse.bass_utils import run_bass_kernel_spmd
from concourse.masks import make_identity, make_upper_triangular

F32 = mybir.dt.float32
F16 = mybir.dt.float16

# Problem constants
B, S, D = 1, 2048, 2048
H, C, HD = 16, 2, 128
DM = HD * C  # 256 per-head q/k dim
N_CORES = 8
HPC = H // N_CORES  # heads per core = 2
NHC = HPC * C  # head-comp blocks per core = 4
EPS = 1e-9
CONST = 10000.0
SQ = 512  # S_q super-tile width

# pool-size knobs (model-tuned)
CFG = {"ph1": 3, "pps": 6, "vpps": 2, "spp": 2, "ep": 12, "qkps": 2,
       "avps": 1, "ost": 6, "comb": 2, "attc": 2}


def build_kernel(s=S, loop_n=1):
    """Build the per-core Bass kernel (SPMD; per-core data differs).

    loop_n > 1 wraps the whole body in a hardware loop (timing only)."""
    import contextlib

    ns = s // 128  # S chunks of 128
    nj = s // SQ  # S_q super tiles
    kd = D // 128  # contraction chunks over D

    nc = bacc.Bacc("TRN2", target_bir_lowering=False, debug=False,
                   num_devices=N_CORES)

    x_d = nc.dram_tensor("x", [s, D], F16, kind="ExternalInput")
    xtr_d = nc.dram_tensor("xtr", [D, s], F16, kind="ExternalInput")
    wqt_d = nc.dram_tensor("wqt", [D, NHC * 128], F16, kind="ExternalInput")
    wkt_d = nc.dram_tensor("wkt", [D, NHC * 128], F16, kind="ExternalInput")
    wvt_d = nc.dram_tensor("wvt", [D, HPC * HD], F16, kind="ExternalInput")
    wot_d = nc.dram_tensor("wot", [HPC * HD, D], F16, kind="ExternalInput")
    cost_d = nc.dram_tensor("cost", [128, s], F16, kind="ExternalInput")
    sint_d = nc.dram_tensor("sint", [128, s], F16, kind="ExternalInput")
    lam_d = nc.dram_tensor("lam", [1, HPC], F32, kind="ExternalInput")
    out_d = nc.dram_tensor("out", [s, D], F16, kind="ExternalOutput")

    inv_sqrt_hd = 1.0 / math.sqrt(HD)
    I32 = mybir.dt.int32
    # float32 whose bit pattern is the rsqrt magic constant 0x5f3759df
    RSQRT_MAGIC = float(np.frombuffer(np.uint32(0x5F3759DF).tobytes(),
                                      np.float32)[0])

    def emit_rsqrt(out_f32, m_f32, ytile, ttile, ktile, shape, eng=None):
        """out = m^-0.5 via bit-trick seed + 2 Newton steps (no tables).
        ytile/ttile are f32 scratch APs of `shape`; ktile holds the magic."""
        eng = eng or nc.vector
        mul = mybir.AluOpType.mult
        eng.tensor_scalar(
            out=ytile.bitcast(I32), in0=m_f32.bitcast(I32), scalar1=1,
            scalar2=None, op0=mybir.AluOpType.logical_shift_right)
        eng.tensor_tensor(
            out=ytile.bitcast(I32), in0=ktile.bitcast(I32).to_broadcast(shape),
            in1=ytile.bitcast(I32), op=mybir.AluOpType.subtract)
        for it in range(2):
            tgt = out_f32 if it == 1 else ytile
            eng.tensor_tensor(out=ttile, in0=ytile, in1=ytile, op=mul)
            eng.tensor_tensor(out=ttile, in0=ttile, in1=m_f32, op=mul)
            eng.tensor_scalar(out=ttile, in0=ttile, scalar1=-0.5,
                              scalar2=1.5, op0=mul,
                              op1=mybir.AluOpType.add)
            eng.tensor_tensor(out=tgt, in0=ytile, in1=ttile, op=mul)

    with tile.TileContext(nc) as tc:
        with (
            (tc.For_i(0, loop_n, 1) if loop_n > 1
             else contextlib.nullcontext()),
            tc.tile_pool(name="const", bufs=1) as cp,
            tc.tile_pool(name="qk", bufs=1) as qkp,
            tc.tile_pool(name="vat", bufs=1) as vap,
        ):
            # ---- small persistent constants ----
            lam = cp.tile([128, HPC], F32, tag="lam")
            m0 = cp.tile([128, 128], F16, tag="m0")
            make_upper_triangular(nc, m0, val=1.0, diag=True)
            ident = cp.tile([128, 128], F16, tag="ident")
            make_identity(nc, ident)
            ktile = cp.tile([128, 1], F32, tag="ktile")
            nc.vector.memset(ktile, RSQRT_MAGIC)

            # persistent activations (split into dependency-granular tiles
            # so consumers start as soon as their slice is ready)
            qTs = [qkp.tile([128, s], F16, tag=f"qT{hc}", name=f"qT{hc}")
                   for hc in range(NHC)]
            kTs = [qkp.tile([128, s], F16, tag=f"kT{hc}", name=f"kT{hc}")
                   for hc in range(NHC)]
            # vaug per (head, S-quarter): [128, 4, 132]
            vaug = [[vap.tile([128, 4, 132], F16, tag=f"va{h}_{q}", name=f"va{h}_{q}")
                     for q in range(nj)] for h in range(HPC)]

            with (
                tc.tile_pool(name="wqkv", bufs=1) as wp,
                tc.tile_pool(name="ht", bufs=1) as htp,
            ):
                wqt = wp.tile([128, kd, NHC * 128], F16, tag="wqt")
                wkt = wp.tile([128, kd, NHC * 128], F16, tag="wkt")
                wvt = wp.tile([128, kd, HPC * HD], F16, tag="wvt")
                # hT split into S-quarters so projections of quarter j only
                # wait on that quarter's 4 transposes
                hts = [htp.tile([128, kd, SQ], F16, tag=f"ht{q}", name=f"ht{q}")
                       for q in range(nj)]

                # ---- phase 1: hT loads (host-pre-transposed x) + stats ----
                # The rmsnorm scale r is deferred off the critical path:
                # r multiplies q/k exactly via the (linear) RoPE tables and
                # v via its per-partition psum eviction. hT arrives as four
                # plain quarter loads of the host-transposed x; the natural
                # x tiles feed only the stats side-path.
                nc.sync.dma_start(out=wvt,
                                  in_=wvt_d.rearrange("(k p) m -> p k m", p=128))
                stats = wp.tile([128, ns], F32, tag="stats")
                rcol = wp.tile([128, ns], F32, tag="rcol")
                xtr_ap = xtr_d.rearrange("(k p) m -> p k m", p=128)
                with (
                    tc.tile_pool(name="ph1", bufs=4) as p1,
                    tc.tile_pool(name="ph1b", bufs=2) as p1b,
                    tc.tile_pool(name="rsc", bufs=2) as rscp,
                ):
                    def stat_group(g):
                        xts = []
                        for i in range(4 * g, 4 * g + 4):
                            xt = p1.tile([128, D], F16, tag="xt")
                            xts.append(xt)
                            nc.sync.dma_start(out=xt, in_=x_d[ts(i, 128), :])
                        for di, i in enumerate(range(4 * g, 4 * g + 4)):
                            sq = p1b.tile([128, D], F16, tag="sq")
                            nc.scalar.activation(
                                out=sq, in_=xts[di],
                                func=mybir.ActivationFunctionType.Square,
                                accum_out=stats[:, i:i + 1])
                        msl = stats[:, 4 * g:4 * g + 4]
                        nc.vector.tensor_scalar(
                            out=msl, in0=msl, scalar1=1.0 / D, scalar2=EPS,
                            op0=mybir.AluOpType.mult, op1=mybir.AluOpType.add)
                        ysc = rscp.tile([128, 4], F32, tag="ysc")
                        tsc = rscp.tile([128, 4], F32, tag="tsc")
                        emit_rsqrt(rcol[:, 4 * g:4 * g + 4], msl, ysc, tsc,
                                   ktile, (128, 4))

                    # interleave: each hT quarter unlocks projection work,
                    # its stats group unlocks the v evictions; q/k weight
                    # chunks slot in after the first/second quarter
                    for g in range(nj):
                        nc.sync.dma_start(out=hts[g],
                                          in_=xtr_ap[:, :, ts(g, SQ)])
                        stat_group(g)
                        if g == 0 or nj == 1:
                            for k in range(kd):
                                nc.sync.dma_start(out=wkt[:, k, :],
                                                  in_=wkt_d[ts(k, 128), :])
                        if g == 1 or nj == 1:
                            for k in range(kd):
                                nc.sync.dma_start(out=wqt[:, k, :],
                                                  in_=wqt_d[ts(k, 128), :])

                # ---- phase 2: projections + RoPE + repack ----
                # q/k are projected into a "split" row layout
                # [R0, R1, I0, I1] (R = rope-real rows, I = rope-imag rows;
                # j2 in {0,1} indexes the two 128-row groups of real parts).
                # RoPE then runs full-lane with partition-aligned operands,
                # and SBUF->SBUF DMAs repack into per-head-comp [xr;xi]
                # tiles (qT/kT) for K=128 attention matmuls.
                mul = mybir.AluOpType.mult
                with tc.tile_pool(name="pps", bufs=CFG["pps"], space="PSUM") as pps, \
                     tc.tile_pool(name="vpps", bufs=CFG["vpps"], space="PSUM") as vpps, \
                     tc.tile_pool(name="split", bufs=CFG["spp"]) as spp, \
                     tc.tile_pool(name="rope", bufs=1) as rp, \
                     tc.tile_pool(name="rdp", bufs=1, space="DRAM") as rdp, \
                     tc.tile_pool(name="ropec", bufs=1) as rcp:
                    cost = rcp.tile([128, s], F16, tag="cost")
                    nc.sync.dma_start(out=cost, in_=cost_d[:, :])
                    sint = rcp.tile([128, s], F16, tag="sint")
                    nc.sync.dma_start(out=sint, in_=sint_d[:, :])
                    # v first so attention's AV operands are ready early;
                    # the deferred rmsnorm scale rides the psum eviction
                    for i in range(ns):
                        ps = vpps.tile([128, HPC * HD], F32, tag="vps")
                        for k in range(kd):
                            nc.tensor.matmul(ps, hts[i // 4][:, k, ts(i % 4, 128)],
                                             wvt[:, k, :],
                                             start=(k == 0), stop=(k == kd - 1))
                        for h in range(HPC):
                            nc.vector.tensor_scalar_mul(
                                out=vaug[h][i // 4][:, i % 4, 0:128],
                                in0=ps[:, ts(h, 128)],
                                scalar1=rcol[:, i:i + 1])
                    for h in range(HPC):
                        for q in range(nj):
                            nc.vector.memset(vaug[h][q][:, :, 128:129], 1.0)
                    # fold r into the rope tables: cos/sin *= r[s] along the
                    # free axis (broadcast r via a DRAM bounce)
                    rc16g = rcp.tile([128, ns], F16, tag="rc16g")
                    nc.vector.tensor_copy(out=rc16g, in_=rcol)
                    rd = rdp.tile([1, s], F16, tag="rd")
                    nc.sync.dma_start(
                        out=rd[0:1, :].rearrange("o (i p) -> o p i", p=128),
                        in_=rc16g)
                    rbc = rcp.tile([128, s], F16, tag="rbc")
                    _rdap = rd[0:1, :]
                    nc.sync.dma_start(
                        out=rbc,
                        in_=bass.AP(tensor=_rdap.tensor, offset=_rdap.offset,
                                    ap=[[0, 128]] + list(_rdap.ap)[1:]))
                    nc.vector.tensor_tensor(out=cost, in0=cost, in1=rbc,
                                            op=mybir.AluOpType.mult)
                    nc.vector.tensor_tensor(out=sint, in0=sint, in1=rbc,
                                            op=mybir.AluOpType.mult)
                    # per (j2, tensor): project the (R_j2, I_j2) pair, RoPE,
                    # repack -- so head j2's attention can start while the
                    # other head is still projecting
                    for j2 in range(2):
                        for w_sb, t_sbs in ((wkt, kTs), (wqt, qTs)):
                            qs2 = spp.tile([128, 2, s], F16, tag="qs")
                            # quarter-major so each hT quarter unlocks both
                            # row-groups' matmuls as soon as it lands
                            for j in range(nj):
                                for mbi, mb in enumerate((j2, j2 + 2)):
                                    ps = pps.tile([128, SQ], F32, tag="ps")
                                    for k in range(kd):
                                        nc.tensor.matmul(
                                            ps, w_sb[:, k, ts(mb, 128)],
                                            hts[j][:, k, :],
                                            start=(k == 0), stop=(k == kd - 1))
                                    nc.scalar.activation(
                                        out=qs2[:, mbi, ts(j, SQ)], in_=ps,
                                        func=mybir.ActivationFunctionType.Copy)
                            # RoPE in place, full 128 lanes
                            xr = qs2[:, 0, :]
                            xi = qs2[:, 1, :]
                            t2 = rp.tile([128, s], F16, tag="t2")
                            t3 = rp.tile([128, s], F16, tag="t3")
                            nc.vector.tensor_tensor(out=t2, in0=xi, in1=sint,
                                                    op=mul)
                            nc.vector.tensor_tensor(out=t3, in0=xr, in1=sint,
                                                    op=mul)
                            nc.vector.tensor_tensor(out=xr, in0=xr, in1=cost,
                                                    op=mul)
                            nc.vector.tensor_tensor(out=xr, in0=xr, in1=t2,
                                                    op=mybir.AluOpType.subtract)
                            nc.vector.tensor_tensor(out=xi, in0=xi, in1=cost,
                                                    op=mul)
                            nc.vector.tensor_tensor(out=xi, in0=xi, in1=t3,
                                                    op=mybir.AluOpType.add)
                            # repack: hc tile = [xr(64) ; xi(64)]
                            for half in range(2):
                                hc = 2 * j2 + half
                                nc.sync.dma_start(
                                    out=t_sbs[hc][0:64, :],
                                    in_=qs2[ts(half, 64), 0, :])
                                nc.sync.dma_start(
                                    out=t_sbs[hc][64:128, :],
                                    in_=qs2[ts(half, 64), 1, :])

            # ---- phase 3 + 4: attention then output projection ----
            # attT per (head, S_q super-tile) so the output projection can
            # start on a row range as soon as both heads' combines finish.
            attT = [[qkp.tile([128, SQ], F16, tag=f"attT{h}_{q}", name=f"attT{h}_{q}")
                     for q in range(nj)] for h in range(HPC)]
            with (
                tc.tile_pool(name="ep", bufs=CFG["ep"]) as ep,
                tc.tile_pool(name="qkps", bufs=CFG["qkps"], space="PSUM") as qkps,
                tc.tile_pool(name="avps", bufs=CFG["avps"], space="PSUM") as avps,
                tc.tile_pool(name="tpps", bufs=1, space="PSUM") as tpps,
                tc.tile_pool(name="comb", bufs=CFG["comb"]) as cbp,
                tc.tile_pool(name="attc", bufs=CFG["attc"]) as atcp,
                tc.tile_pool(name="small", bufs=8) as smp,
                tc.tile_pool(name="wo", bufs=1) as wop,
                tc.tile_pool(name="ops", bufs=2, space="PSUM") as opsp,
                tc.tile_pool(name="ost", bufs=CFG["ost"]) as ostp,
            ):
                wot = wop.tile([128, HPC, D], F16, tag="wot")
                nc.sync.dma_start(out=wot,
                                  in_=wot_d.rearrange("(h p) n -> p h n", p=128))
                _lap = lam_d[:, :]
                nc.sync.dma_start(
                    out=lam,
                    in_=bass.AP(tensor=_lap.tensor, offset=_lap.offset,
                                ap=[[0, 128]] + list(_lap.ap)[1:]),
                )
                mul = mybir.AluOpType.mult
                add = mybir.AluOpType.add
                for j in range(nj - 1, -1, -1):
                    for head in range(HPC):
                        avsb = []
                        for c2 in range(C):
                            hc = C * head + c2
                            nblk = 4 * j + 4
                            # fused softmax normalization targets
                            attn_c = atcp.tile([128, 4, 128], F16,
                                               tag=f"attn{c2}", name=f"attn{c2}")
                            drc = smp.tile([128, 4, 1], F32, tag=f"drc{c2}",
                                           name=f"drc{c2}")
                            avsb.append(attn_c)
                            es = []
                            # S_k blocks in pairs: one 2-bank PSUM tile and
                            # (off-diagonal) one wide Exp per pair
                            for i2 in range(0, nblk, 2):
                                eps2 = qkps.tile([128, 2, SQ], F32, tag="eps")
                                et2 = ep.tile([128, 2, SQ], F16, tag="et")
                                diag = i2 >= 4 * j
                                for di in range(2):
                                    i = i2 + di
                                    c0 = 128 * max(i - 4 * j, 0)
                                    nc.tensor.matmul(
                                        eps2[:, di, c0:SQ], kTs[hc][:, ts(i, 128)],
                                        qTs[hc][:, SQ * j + c0:SQ * j + SQ],
                                        start=True, stop=True)
                                    if diag:
                                        nc.scalar.activation(
                                            out=et2[:, di, c0:SQ],
                                            in_=eps2[:, di, c0:SQ],
                                            func=mybir.ActivationFunctionType.Exp,
                                            scale=inv_sqrt_hd)
                                        nc.vector.tensor_tensor(
                                            out=et2[:, di, c0:c0 + 128],
                                            in0=et2[:, di, c0:c0 + 128],
                                            in1=m0, op=mul)
                                if not diag:
                                    nc.scalar.activation(
                                        out=et2, in_=eps2,
                                        func=mybir.ActivationFunctionType.Exp,
                                        scale=inv_sqrt_hd)
                                es.append(et2)
                            for m in range(4):
                                avm = avps.tile([128, 129], F32, tag="avm")
                                for i in range(4 * j + m + 1):
                                    nc.tensor.matmul(
                                        avm, es[i // 2][:, i % 2, ts(m, 128)],
                                        vaug[head][i // 4][:, i % 4, 0:129],
                                        start=(i == 0), stop=(i == 4 * j + m))
                                nc.vector.reciprocal(out=drc[:, m, :],
                                                     in_=avm[:, 128:129])
                                nc.vector.tensor_scalar_mul(
                                    out=attn_c[:, m, :], in0=avm[:, 0:128],
                                    scalar1=drc[:, m, :])
                        # combine components + head RMSNorm (f16, 2x mode)
                        comb = cbp.tile([128, 4, 128], F16, tag="comb")
                        nc.vector.scalar_tensor_tensor(
                            out=comb, in0=avsb[1], scalar=lam[:, head:head + 1],
                            in1=avsb[0], op0=mul, op1=add)
                        tt = cbp.tile([128, 4, 128], F16, tag="tt")
                        nc.vector.tensor_tensor(out=tt, in0=comb, in1=comb, op=mul)
                        ssum = smp.tile([128, 4, 1], F32, tag="ssum")
                        nc.vector.reduce_sum(out=ssum, in_=tt,
                                             axis=mybir.AxisListType.X)
                        nc.vector.tensor_scalar(
                            out=ssum, in0=ssum, scalar1=1.0 / HD, scalar2=EPS,
                            op0=mul, op1=add)
                        rf = smp.tile([128, 4, 1], F32, tag="rf")
                        ycb = smp.tile([128, 4, 1], F32, tag="ycb")
                        tcb = smp.tile([128, 4, 1], F32, tag="tcb")
                        emit_rsqrt(rf, ssum, ycb, tcb,
                                   ktile[:, :, None], (128, 4, 1))
                        a16 = cbp.tile([128, 4, 128], F16, tag="a16")
                        nc.vector.tensor_tensor(
                            out=a16, in0=comb, in1=rf.to_broadcast((128, 4, 128)),
                            op=mul)
                        for mm in range(4):
                            tp = tpps.tile([128, 128], F16, tag="tp")
                            nc.tensor.transpose(tp, a16[:, mm, :], ident)
                            nc.vector.tensor_copy(
                                out=attT[head][j][:, ts(mm, 128)], in_=tp)
                        if head == HPC - 1:
                            # output projection for this super-tile's rows
                            for sm in range(4 * j, 4 * j + 4):
                                for dn in range(D // SQ):
                                    ps = opsp.tile([128, SQ], F32, tag="ops")
                                    for h in range(HPC):
                                        nc.tensor.matmul(
                                            ps,
                                            attT[h][sm // 4][:, ts(sm % 4, 128)],
                                            wot[:, h, ts(dn, SQ)],
                                            start=(h == 0), stop=(h == HPC - 1))
                                    ost = ostp.tile([128, SQ], F16, tag="ost")
                                    if j <= 1:
                                        nc.scalar.activation(
                                            out=ost, in_=ps,
                                            func=mybir.ActivationFunctionType.Copy)
                                    else:
                                        nc.vector.tensor_copy(out=ost, in_=ps)
                                    nc.sync.dma_start(
                                        out=out_d[ts(sm, 128), ts(dn, SQ)],
                                        in_=ost)

    nc.compile()
    return nc


def _perm_core():
    """Row permutation of one core's HPC*DM q/k rows into the split layout
    [R0..R_{HPC-1}, I0..I_{HPC-1}]: R_h = rope-real (even) rows of head h for
    both components, I_h = rope-imag (odd) rows. Within each 128-row block,
    rows follow theta-pair order 0..127."""
    evens = [h * DM + 128 * c + 2 * t
             for h in range(HPC) for c in range(C) for t in range(64)]
    odds = [h * DM + 128 * c + 2 * t + 1
            for h in range(HPC) for c in range(C) for t in range(64)]
    return np.array(evens + odds)


def prep_inputs(x, pre_norm_w, wq, wk, wv, wo, head_norm_w, q1, q2, k1, k2,
                lam_init, s=S):
    """Host-side prep: fold norms/lambdas into weights, permute q/k rows,
    transpose, cast fp16, build rope tables; returns per-core input maps."""
    x2 = np.asarray(x, np.float32).reshape(s, D)
    pw = np.asarray(pre_norm_w, np.float32)
    hw = np.asarray(head_norm_w, np.float32)
    li = np.asarray(lam_init, np.float64)

    wq_e = (np.asarray(wq, np.float64) * pw[None, :])
    wk_e = (np.asarray(wk, np.float64) * pw[None, :])
    wv_e = (np.asarray(wv, np.float64) * pw[None, :])
    # wo: out = att_normed * (1-lam) @ wo.T ; head_norm_w folds per att dim
    colscale = np.concatenate(
        [hw.astype(np.float64) * (1.0 - li[h]) for h in range(H)])
    wo_e = np.asarray(wo, np.float64) * colscale[None, :]

    base = (np.exp(np.sum(np.asarray(q1, np.float64) * np.asarray(k1, np.float64),
                          axis=-2))
            - np.exp(np.sum(np.asarray(q2, np.float64) * np.asarray(k2, np.float64),
                            axis=-2)))  # (H, 1)
    scale_h = -(H * base[:, 0] + li.sum())  # (H,)

    theta = 1.0 / (CONST ** (np.arange(0, DM, 2, dtype=np.float64) / DM))
    ang = np.arange(s, dtype=np.float64)[:, None] * theta[None, :]  # (s, 128)
    cost = np.cos(ang).T.astype(np.float16)  # (128, s)
    sint = np.sin(ang).T.astype(np.float16)

    x16 = x2.astype(np.float16)
    xtr = np.ascontiguousarray(x16.T)
    ph = _perm_core()
    in_maps = []
    for core in range(N_CORES):
        heads = range(core * HPC, (core + 1) * HPC)
        rows = core * HPC * DM + ph
        wqt = np.ascontiguousarray(wq_e[rows].T).astype(np.float16)
        wkt = np.ascontiguousarray(wk_e[rows].T).astype(np.float16)
        vrows = np.concatenate(
            [np.arange(h * HD, (h + 1) * HD) for h in heads])
        wvt = np.ascontiguousarray(wv_e[vrows].T).astype(np.float16)
        wot = np.ascontiguousarray(wo_e[:, vrows].T).astype(np.float16)
        lamc = scale_h[list(heads)].astype(np.float32).reshape(1, HPC)
        in_maps.append({
            "x": x16, "xtr": xtr, "wqt": wqt, "wkt": wkt, "wvt": wvt,
            "wot": wot, "cost": cost, "sint": sint, "lam": lamc,
        })
    return in_maps


_NC_CACHE = {}


def kernel(x, pre_norm_w, wq, wk, wv, wo, head_norm_w, q1, q2, k1, k2,
           lam_init):
    s = x.shape[1]
    if s not in _NC_CACHE:
        _NC_CACHE[s] = build_kernel(s)
    nc = _NC_CACHE[s]
    in_maps = prep_inputs(x, pre_norm_w, wq, wk, wv, wo, head_norm_w,
                          q1, q2, k1, k2, lam_init, s=s)
    res = run_bass_kernel_spmd(nc, in_maps, list(range(N_CORES)))
    acc = np.zeros((s, D), np.float64)
    for c in range(N_CORES):
        acc += res.results[c]["out"].astype(np.float64)
    out = acc.astype(np.float32) + np.asarray(x, np.float32).reshape(s, D)
    return out.reshape(1, s, D)


# revision 73
# speedup vs baseline: 1.0717x; 1.0099x over previous
"""Differential attention (dense_transformer) Trainium2 kernel.

Full-input contract: kernel(**inputs) takes the unsharded inputs of
reference.setup_inputs() and returns the full (1, S, D) float32 output.

Sharding: 16 heads across 8 cores (2 heads/core, tensor-parallel on the
q/k/v projection rows and wo columns). Each core computes a full (S, D)
partial of the output projection; the host sums partials and adds the
residual.
"""

import sys

for _p in ("/opt/trn_rl_repo", "/root/.axon_site/_ro/trn_rl_repo"):
    if _p not in sys.path:
        sys.path.insert(0, _p)

import math

import numpy as np

import concourse.bass as bass
import concourse.mybir as mybir
import concourse.tile as tile
from concourse import bacc
from concourse.bass import ts
from concourse.bass_utils import run_bass_kernel_spmd
from concourse.masks import make_identity, make_upper_triangular

F32 = mybir.dt.float32
F16 = mybir.dt.float16

# Problem constants
B, S, D = 1, 2048, 2048
H, C, HD = 16, 2, 128
DM = HD * C  # 256 per-head q/k dim
N_CORES = 8
HPC = H // N_CORES  # heads per core = 2
NHC = HPC * C  # head-comp blocks per core = 4
EPS = 1e-9
CONST = 10000.0
SQ = 512  # S_q super-tile width

# pool-size knobs (model-tuned)
CFG = {"ph1": 3, "pps": 6, "vpps": 2, "spp": 2, "ep": 12, "qkps": 2,
       "avps": 1, "ost": 6, "comb": 2, "attc": 2}


def build_kernel(s=S, loop_n=1):
    """Build the per-core Bass kernel (SPMD; per-core data differs).

    loop_n > 1 wraps the whole body in a hardware loop (timing only)."""
    import contextlib

    ns = s // 128  # S chunks of 128
    nj = s // SQ  # S_q super tiles
    kd = D // 128  # contraction chunks over D

    nc = bacc.Bacc("TRN2", target_bir_lowering=False, debug=False,
                   num_devices=N_CORES)

    x_d = nc.dram_tensor("x", [s, D], F16, kind="ExternalInput")
    xtr_d = nc.dram_tensor("xtr", [D, s], F16, kind="ExternalInput")
    wqt_d = nc.dram_tensor("wqt", [D, NHC * 128], F16, kind="ExternalInput")
    wkt_d = nc.dram_tensor("wkt", [D, NHC * 128], F16, kind="ExternalInput")
    wvt_d = nc.dram_tensor("wvt", [D, HPC * HD], F16, kind="ExternalInput")
    wot_d = nc.dram_tensor("wot", [HPC * HD, D], F16, kind="ExternalInput")
    cost_d = nc.dram_tensor("cost", [128, s], F16, kind="ExternalInput")
    sint_d = nc.dram_tensor("sint", [128, s], F16, kind="ExternalInput")
    lam_d = nc.dram_tensor("lam", [1, HPC], F32, kind="ExternalInput")
    out_d = nc.dram_tensor("out", [s, D], F16, kind="ExternalOutput")

    inv_sqrt_hd = 1.0 / math.sqrt(HD)
    I32 = mybir.dt.int32
    # float32 whose bit pattern is the rsqrt magic constant 0x5f3759df
    RSQRT_MAGIC = float(np.frombuffer(np.uint32(0x5F3759DF).tobytes(),
                                      np.float32)[0])

    def emit_rsqrt(out_f32, m_f32, ytile, ttile, ktile, shape, eng=None):
        """out = m^-0.5 via bit-trick seed + 2 Newton steps (no tables).
        ytile/ttile are f32 scratch APs of `shape`; ktile holds the magic."""
        eng = eng or nc.vector
        mul = mybir.AluOpType.mult
        eng.tensor_scalar(
            out=ytile.bitcast(I32), in0=m_f32.bitcast(I32), scalar1=1,
            scalar2=None, op0=mybir.AluOpType.logical_shift_right)
        eng.tensor_tensor(
            out=ytile.bitcast(I32), in0=ktile.bitcast(I32).to_broadcast(shape),
            in1=ytile.bitcast(I32), op=mybir.AluOpType.subtract)
        for it in range(2):
            tgt = out_f32 if it == 1 else ytile
            eng.tensor_tensor(out=ttile, in0=ytile, in1=ytile, op=mul)
            eng.tensor_tensor(out=ttile, in0=ttile, in1=m_f32, op=mul)
            eng.tensor_scalar(out=ttile, in0=ttile, scalar1=-0.5,
                              scalar2=1.5, op0=mul,
                              op1=mybir.AluOpType.add)
            eng.tensor_tensor(out=tgt, in0=ytile, in1=ttile, op=mul)

    with tile.TileContext(nc) as tc:
        with (
            (tc.For_i(0, loop_n, 1) if loop_n > 1
             else contextlib.nullcontext()),
            tc.tile_pool(name="const", bufs=1) as cp,
            tc.tile_pool(name="qk", bufs=1) as qkp,
            tc.tile_pool(name="vat", bufs=1) as vap,
        ):
            # ---- small persistent constants ----
            lam = cp.tile([128, HPC], F32, tag="lam")
            m0 = cp.tile([128, 128], F16, tag="m0")
            make_upper_triangular(nc, m0, val=1.0, diag=True)
            ident = cp.tile([128, 128], F16, tag="ident")
            make_identity(nc, ident)
            ktile = cp.tile([128, 1], F32, tag="ktile")
            nc.vector.memset(ktile, RSQRT_MAGIC)

            # persistent activations (split into dependency-granular tiles
            # so consumers start as soon as their slice is ready)
            qTs = [qkp.tile([128, s], F16, tag=f"qT{hc}", name=f"qT{hc}")
                   for hc in range(NHC)]
            kTs = [qkp.tile([128, s], F16, tag=f"kT{hc}", name=f"kT{hc}")
                   for hc in range(NHC)]
            # vaug per (head, S-quarter): [128, 4, 132]
            vaug = [[vap.tile([128, 4, 132], F16, tag=f"va{h}_{q}", name=f"va{h}_{q}")
                     for q in range(nj)] for h in range(HPC)]

            with (
                tc.tile_pool(name="wqkv", bufs=1) as wp,
                tc.tile_pool(name="ht", bufs=1) as htp,
            ):
                wqt = wp.tile([128, kd, NHC * 128], F16, tag="wqt")
                wkt = wp.tile([128, kd, NHC * 128], F16, tag="wkt")
                wvt = wp.tile([128, kd, HPC * HD], F16, tag="wvt")
                # hT split into S-quarters so projections of quarter j only
                # wait on that quarter's 4 transposes
                hts = [htp.tile([128, kd, SQ], F16, tag=f"ht{q}", name=f"ht{q}")
                       for q in range(nj)]

                # ---- phase 1: hT loads (host-pre-transposed x) + stats ----
                # The rmsnorm scale r is deferred off the critical path:
                # r multiplies q/k exactly via the (linear) RoPE tables and
                # v via its per-partition psum eviction. hT arrives as four
                # plain quarter loads of the host-transposed x; the natural
                # x tiles feed only the stats side-path.
                nc.sync.dma_start(out=wvt,
                                  in_=wvt_d.rearrange("(k p) m -> p k m", p=128))
                stats = wp.tile([128, ns], F32, tag="stats")
                rcol = wp.tile([128, ns], F32, tag="rcol")
                xtr_ap = xtr_d.rearrange("(k p) m -> p k m", p=128)
                with (
                    tc.tile_pool(name="ph1", bufs=4) as p1,
                    tc.tile_pool(name="ph1b", bufs=2) as p1b,
                    tc.tile_pool(name="rsc", bufs=2) as rscp,
                ):
                    def stat_group(g):
                        xts = []
                        for i in range(4 * g, 4 * g + 4):
                            xt = p1.tile([128, D], F16, tag="xt")
                            xts.append(xt)
                            nc.sync.dma_start(out=xt, in_=x_d[ts(i, 128), :])
                        for di, i in enumerate(range(4 * g, 4 * g + 4)):
                            sq = p1b.tile([128, D], F16, tag="sq")
                            nc.scalar.activation(
                                out=sq, in_=xts[di],
                                func=mybir.ActivationFunctionType.Square,
                                accum_out=stats[:, i:i + 1])
                        msl = stats[:, 4 * g:4 * g + 4]
                        nc.vector.tensor_scalar(
                            out=msl, in0=msl, scalar1=1.0 / D, scalar2=EPS,
                            op0=mybir.AluOpType.mult, op1=mybir.AluOpType.add)
                        ysc = rscp.tile([128, 4], F32, tag="ysc")
                        tsc = rscp.tile([128, 4], F32, tag="tsc")
                        emit_rsqrt(rcol[:, 4 * g:4 * g + 4], msl, ysc, tsc,
                                   ktile, (128, 4))

                    # interleave: each hT quarter unlocks projection work,
                    # its stats group unlocks the v evictions; q/k weight
                    # chunks slot in after the first/second quarter
                    for g in range(nj):
                        nc.sync.dma_start(out=hts[g],
                                          in_=xtr_ap[:, :, ts(g, SQ)])
                        stat_group(g)
                        if g == 0 or nj == 1:
                            for k in range(kd):
                                nc.sync.dma_start(out=wkt[:, k, :],
                                                  in_=wkt_d[ts(k, 128), :])
                        if g == 1 or nj == 1:
                            for k in range(kd):
                                nc.sync.dma_start(out=wqt[:, k, :],
                                                  in_=wqt_d[ts(k, 128), :])

                # ---- phase 2: projections + RoPE + repack ----
                # q/k are projected into a "split" row layout
                # [R0, R1, I0, I1] (R = rope-real rows, I = rope-imag rows;
                # j2 in {0,1} indexes the two 128-row groups of real parts).
                # RoPE then runs full-lane with partition-aligned operands,
                # and SBUF->SBUF DMAs repack into per-head-comp [xr;xi]
                # tiles (qT/kT) for K=128 attention matmuls.
                mul = mybir.AluOpType.mult
                with tc.tile_pool(name="pps", bufs=CFG["pps"], space="PSUM") as pps, \
                     tc.tile_pool(name="vpps", bufs=CFG["vpps"], space="PSUM") as vpps, \
                     tc.tile_pool(name="split", bufs=CFG["spp"]) as spp, \
                     tc.tile_pool(name="rope", bufs=1) as rp, \
                     tc.tile_pool(name="rdp", bufs=1, space="DRAM") as rdp, \
                     tc.tile_pool(name="ropec", bufs=1) as rcp:
                    cost = rcp.tile([128, s], F16, tag="cost")
                    nc.sync.dma_start(out=cost, in_=cost_d[:, :])
                    sint = rcp.tile([128, s], F16, tag="sint")
                    nc.sync.dma_start(out=sint, in_=sint_d[:, :])
                    # v first so attention's AV operands are ready early;
                    # the deferred rmsnorm scale rides the psum eviction
                    for i in range(ns):
                        ps = vpps.tile([128, HPC * HD], F32, tag="vps")
                        for k in range(kd):
                            nc.tensor.matmul(ps, hts[i // 4][:, k, ts(i % 4, 128)],
                                             wvt[:, k, :],
                                             start=(k == 0), stop=(k == kd - 1))
                        for h in range(HPC):
                            nc.vector.tensor_scalar_mul(
                                out=vaug[h][i // 4][:, i % 4, 0:128],
                                in0=ps[:, ts(h, 128)],
                                scalar1=rcol[:, i:i + 1])
                    for h in range(HPC):
                        for q in range(nj):
                            nc.vector.memset(vaug[h][q][:, :, 128:129], 1.0)
                    # fold r into the rope tables: cos/sin *= r[s] along the
                    # free axis (broadcast r via a DRAM bounce)
                    rc16g = rcp.tile([128, ns], F16, tag="rc16g")
                    nc.vector.tensor_copy(out=rc16g, in_=rcol)
                    rd = rdp.tile([1, s], F16, tag="rd")
                    nc.sync.dma_start(
                        out=rd[0:1, :].rearrange("o (i p) -> o p i", p=128),
                        in_=rc16g)
                    rbc = rcp.tile([128, s], F16, tag="rbc")
                    _rdap = rd[0:1, :]
                    nc.sync.dma_start(
                        out=rbc,
                        in_=bass.AP(tensor=_rdap.tensor, offset=_rdap.offset,
                                    ap=[[0, 128]] + list(_rdap.ap)[1:]))
                    nc.vector.tensor_tensor(out=cost, in0=cost, in1=rbc,
                                            op=mybir.AluOpType.mult)
                    nc.vector.tensor_tensor(out=sint, in0=sint, in1=rbc,
                                            op=mybir.AluOpType.mult)
                    # per (j2, tensor): project the (R_j2, I_j2) pair, RoPE,
                    # repack -- so head j2's attention can start while the
                    # other head is still projecting
                    for j2 in range(2):
                        for w_sb, t_sbs in ((wkt, kTs), (wqt, qTs)):
                            qs2 = spp.tile([128, 2, s], F16, tag="qs")
                            # quarter-major so each hT quarter unlocks both
                            # row-groups' matmuls as soon as it lands
                            for j in range(nj):
                                for mbi, mb in enumerate((j2, j2 + 2)):
                                    ps = pps.tile([128, SQ], F32, tag="ps")
                                    for k in range(kd):
                                        nc.tensor.matmul(
                                            ps, w_sb[:, k, ts(mb, 128)],
                                            hts[j][:, k, :],
                                            start=(k == 0), stop=(k == kd - 1))
                                    nc.scalar.activation(
                                        out=qs2[:, mbi, ts(j, SQ)], in_=ps,
                                        func=mybir.ActivationFunctionType.Copy)
                            # RoPE in place, full 128 lanes
                            xr = qs2[:, 0, :]
                            xi = qs2[:, 1, :]
                            t2 = rp.tile([128, s], F16, tag="t2")
                            t3 = rp.tile([128, s], F16, tag="t3")
                            nc.vector.tensor_tensor(out=t2, in0=xi, in1=sint,
                                                    op=mul)
                            nc.vector.tensor_tensor(out=t3, in0=xr, in1=sint,
                                                    op=mul)
                            nc.vector.tensor_tensor(out=xr, in0=xr, in1=cost,
                                                    op=mul)
                            nc.vector.tensor_tensor(out=xr, in0=xr, in1=t2,
                                                    op=mybir.AluOpType.subtract)
                            nc.vector.tensor_tensor(out=xi, in0=xi, in1=cost,
                                                    op=mul)
                            nc.vector.tensor_tensor(out=xi, in0=xi, in1=t3,
                                                    op=mybir.AluOpType.add)
                            # repack: hc tile = [xr(64) ; xi(64)]
                            for half in range(2):
                                hc = 2 * j2 + half
                                nc.sync.dma_start(
                                    out=t_sbs[hc][0:64, :],
                                    in_=qs2[ts(half, 64), 0, :])
                                nc.sync.dma_start(
                                    out=t_sbs[hc][64:128, :],
                                    in_=qs2[ts(half, 64), 1, :])

            # ---- phase 3 + 4: attention then output projection ----
            # attT per (head, S_q super-tile) so the output projection can
            # start on a row range as soon as both heads' combines finish.
            attT = [[qkp.tile([128, SQ], F16, tag=f"attT{h}_{q}", name=f"attT{h}_{q}")
                     for q in range(nj)] for h in range(HPC)]
            with (
                tc.tile_pool(name="ep", bufs=CFG["ep"]) as ep,
                tc.tile_pool(name="qkps", bufs=CFG["qkps"], space="PSUM") as qkps,
                tc.tile_pool(name="avps", bufs=CFG["avps"], space="PSUM") as avps,
                tc.tile_pool(name="tpps", bufs=1, space="PSUM") as tpps,
                tc.tile_pool(name="comb", bufs=CFG["comb"]) as cbp,
                tc.tile_pool(name="attc", bufs=CFG["attc"]) as atcp,
                tc.tile_pool(name="small", bufs=8) as smp,
                tc.tile_pool(name="wo", bufs=1) as wop,
                tc.tile_pool(name="ops", bufs=2, space="PSUM") as opsp,
                tc.tile_pool(name="ost", bufs=CFG["ost"]) as ostp,
            ):
                wot = wop.tile([128, HPC, D], F16, tag="wot")
                nc.sync.dma_start(out=wot,
                                  in_=wot_d.rearrange("(h p) n -> p h n", p=128))
                _lap = lam_d[:, :]
                nc.sync.dma_start(
                    out=lam,
                    in_=bass.AP(tensor=_lap.tensor, offset=_lap.offset,
                                ap=[[0, 128]] + list(_lap.ap)[1:]),
                )
                mul = mybir.AluOpType.mult
                add = mybir.AluOpType.add
                for j in range(nj - 1, -1, -1):
                    for head in range(HPC):
                        avsb = []
                        for c2 in range(C):
                            hc = C * head + c2
                            nblk = 4 * j + 4
                            # fused softmax normalization targets
                            attn_c = atcp.tile([128, 4, 128], F16,
                                               tag=f"attn{c2}", name=f"attn{c2}")
                            drc = smp.tile([128, 4, 1], F32, tag=f"drc{c2}",
                                           name=f"drc{c2}")
                            avsb.append(attn_c)
                            es = []
                            # S_k blocks in pairs: one 2-bank PSUM tile and
                            # (off-diagonal) one wide Exp per pair
                            for i2 in range(0, nblk, 2):
                                eps2 = qkps.tile([128, 2, SQ], F32, tag="eps")
                                et2 = ep.tile([128, 2, SQ], F16, tag="et")
                                diag = i2 >= 4 * j
                                for di in range(2):
                                    i = i2 + di
                                    c0 = 128 * max(i - 4 * j, 0)
                                    nc.tensor.matmul(
                                        eps2[:, di, c0:SQ], kTs[hc][:, ts(i, 128)],
                                        qTs[hc][:, SQ * j + c0:SQ * j + SQ],
                                        start=True, stop=True)
                                    if diag:
                                        nc.scalar.activation(
                                            out=et2[:, di, c0:SQ],
                                            in_=eps2[:, di, c0:SQ],
                                            func=mybir.ActivationFunctionType.Exp,
                                            scale=inv_sqrt_hd)
                                        nc.gpsimd.tensor_tensor(
                                            out=et2[:, di, c0:c0 + 128],
                                            in0=et2[:, di, c0:c0 + 128],
                                            in1=m0, op=mul)
                                if not diag:
                                    nc.scalar.activation(
                                        out=et2, in_=eps2,
                                        func=mybir.ActivationFunctionType.Exp,
                                        scale=inv_sqrt_hd)
                                es.append(et2)
                            for m in range(4):
                                avm = avps.tile([128, 129], F32, tag="avm")
                                for i in range(4 * j + m + 1):
                                    nc.tensor.matmul(
                                        avm, es[i // 2][:, i % 2, ts(m, 128)],
                                        vaug[head][i // 4][:, i % 4, 0:129],
                                        start=(i == 0), stop=(i == 4 * j + m))
                                nc.vector.reciprocal(out=drc[:, m, :],
                                                     in_=avm[:, 128:129])
                                nc.vector.tensor_scalar_mul(
                                    out=attn_c[:, m, :], in0=avm[:, 0:128],
                                    scalar1=drc[:, m, :])
                        # combine components + head RMSNorm (f16, 2x mode)
                        comb = cbp.tile([128, 4, 128], F16, tag="comb")
                        nc.vector.scalar_tensor_tensor(
                            out=comb, in0=avsb[1], scalar=lam[:, head:head + 1],
                            in1=avsb[0], op0=mul, op1=add)
                        tt = cbp.tile([128, 4, 128], F16, tag="tt")
                        nc.vector.tensor_tensor(out=tt, in0=comb, in1=comb, op=mul)
                        ssum = smp.tile([128, 4, 1], F32, tag="ssum")
                        nc.vector.reduce_sum(out=ssum, in_=tt,
                                             axis=mybir.AxisListType.X)
                        nc.vector.tensor_scalar(
                            out=ssum, in0=ssum, scalar1=1.0 / HD, scalar2=EPS,
                            op0=mul, op1=add)
                        rf = smp.tile([128, 4, 1], F32, tag="rf")
                        ycb = smp.tile([128, 4, 1], F32, tag="ycb")
                        tcb = smp.tile([128, 4, 1], F32, tag="tcb")
                        emit_rsqrt(rf, ssum, ycb, tcb,
                                   ktile[:, :, None], (128, 4, 1))
                        a16 = cbp.tile([128, 4, 128], F16, tag="a16")
                        nc.vector.tensor_tensor(
                            out=a16, in0=comb, in1=rf.to_broadcast((128, 4, 128)),
                            op=mul)
                        for mm in range(4):
                            tp = tpps.tile([128, 128], F16, tag="tp")
                            nc.tensor.transpose(tp, a16[:, mm, :], ident)
                            nc.vector.tensor_copy(
                                out=attT[head][j][:, ts(mm, 128)], in_=tp)
                        if head == HPC - 1:
                            # output projection for this super-tile's rows
                            for sm in range(4 * j, 4 * j + 4):
                                for dn in range(D // SQ):
                                    ps = opsp.tile([128, SQ], F32, tag="ops")
                                    for h in range(HPC):
                                        nc.tensor.matmul(
                                            ps,
                                            attT[h][sm // 4][:, ts(sm % 4, 128)],
                                            wot[:, h, ts(dn, SQ)],
                                            start=(h == 0), stop=(h == HPC - 1))
                                    ost = ostp.tile([128, SQ], F16, tag="ost")
                                    if j <= 1:
                                        nc.scalar.activation(
                                            out=ost, in_=ps,
                                            func=mybir.ActivationFunctionType.Copy)
                                    else:
                                        nc.vector.tensor_copy(out=ost, in_=ps)
                                    nc.sync.dma_start(
                                        out=out_d[ts(sm, 128), ts(dn, SQ)],
                                        in_=ost)

    nc.compile()
    return nc


def _perm_core():
    """Row permutation of one core's HPC*DM q/k rows into the split layout
    [R0..R_{HPC-1}, I0..I_{HPC-1}]: R_h = rope-real (even) rows of head h for
    both components, I_h = rope-imag (odd) rows. Within each 128-row block,
    rows follow theta-pair order 0..127."""
    evens = [h * DM + 128 * c + 2 * t
             for h in range(HPC) for c in range(C) for t in range(64)]
    odds = [h * DM + 128 * c + 2 * t + 1
            for h in range(HPC) for c in range(C) for t in range(64)]
    return np.array(evens + odds)


def prep_inputs(x, pre_norm_w, wq, wk, wv, wo, head_norm_w, q1, q2, k1, k2,
                lam_init, s=S):
    """Host-side prep: fold norms/lambdas into weights, permute q/k rows,
    transpose, cast fp16, build rope tables; returns per-core input maps."""
    x2 = np.asarray(x, np.float32).reshape(s, D)
    pw = np.asarray(pre_norm_w, np.float32)
    hw = np.asarray(head_norm_w, np.float32)
    li = np.asarray(lam_init, np.float64)

    wq_e = (np.asarray(wq, np.float64) * pw[None, :])
    wk_e = (np.asarray(wk, np.float64) * pw[None, :])
    wv_e = (np.asarray(wv, np.float64) * pw[None, :])
    # wo: out = att_normed * (1-lam) @ wo.T ; head_norm_w folds per att dim
    colscale = np.concatenate(
        [hw.astype(np.float64) * (1.0 - li[h]) for h in range(H)])
    wo_e = np.asarray(wo, np.float64) * colscale[None, :]

    base = (np.exp(np.sum(np.asarray(q1, np.float64) * np.asarray(k1, np.float64),
                          axis=-2))
            - np.exp(np.sum(np.asarray(q2, np.float64) * np.asarray(k2, np.float64),
                            axis=-2)))  # (H, 1)
    scale_h = -(H * base[:, 0] + li.sum())  # (H,)

    theta = 1.0 / (CONST ** (np.arange(0, DM, 2, dtype=np.float64) / DM))
    ang = np.arange(s, dtype=np.float64)[:, None] * theta[None, :]  # (s, 128)
    cost = np.cos(ang).T.astype(np.float16)  # (128, s)
    sint = np.sin(ang).T.astype(np.float16)

    x16 = x2.astype(np.float16)
    xtr = np.ascontiguousarray(x16.T)
    ph = _perm_core()
    in_maps = []
    for core in range(N_CORES):
        heads = range(core * HPC, (core + 1) * HPC)
        rows = core * HPC * DM + ph
        wqt = np.ascontiguousarray(wq_e[rows].T).astype(np.float16)
        wkt = np.ascontiguousarray(wk_e[rows].T).astype(np.float16)
        vrows = np.concatenate(
            [np.arange(h * HD, (h + 1) * HD) for h in heads])
        wvt = np.ascontiguousarray(wv_e[vrows].T).astype(np.float16)
        wot = np.ascontiguousarray(wo_e[:, vrows].T).astype(np.float16)
        lamc = scale_h[list(heads)].astype(np.float32).reshape(1, HPC)
        in_maps.append({
            "x": x16, "xtr": xtr, "wqt": wqt, "wkt": wkt, "wvt": wvt,
            "wot": wot, "cost": cost, "sint": sint, "lam": lamc,
        })
    return in_maps


_NC_CACHE = {}


def kernel(x, pre_norm_w, wq, wk, wv, wo, head_norm_w, q1, q2, k1, k2,
           lam_init):
    s = x.shape[1]
    if s not in _NC_CACHE:
        _NC_CACHE[s] = build_kernel(s)
    nc = _NC_CACHE[s]
    in_maps = prep_inputs(x, pre_norm_w, wq, wk, wv, wo, head_norm_w,
                          q1, q2, k1, k2, lam_init, s=s)
    res = run_bass_kernel_spmd(nc, in_maps, list(range(N_CORES)))
    acc = np.zeros((s, D), np.float64)
    for c in range(N_CORES):
        acc += res.results[c]["out"].astype(np.float64)
    out = acc.astype(np.float32) + np.asarray(x, np.float32).reshape(s, D)
    return out.reshape(1, s, D)
